# revision 52
# baseline (speedup 1.0000x reference)
"""Trainium2 Bass kernel for nn_Block_local (dual global/banded-local attention block).

Sharding: data-parallel, one batch element per NeuronCore (B=8, 8 cores).
Feature-major activations ([C,N]); fp8e4 DoubleRow matmuls for all
weight-contractions (weights quantized host-side, fc1/fc2 split hi+lo fp8),
bf16 scores, fp8 softmax/activation intermediates, feature-major banded local
attention (shifts are free-axis slices; no shift DMAs, no local transposes).
"""
import os
import numpy as np
import ml_dtypes

import concourse.bass as bass
import concourse.bacc as bacc
import concourse.mybir as mybir
import concourse.tile as tile
from concourse.bass_utils import run_bass_kernel_spmd
from concourse.masks import make_identity
from contextlib import ExitStack

F32 = mybir.dt.float32
F32R = mybir.dt.float32r
BF16 = mybir.dt.bfloat16
FP8 = mybir.dt.float8e4
AF = mybir.ActivationFunctionType
ALU = mybir.AluOpType
DR = mybir.MatmulPerfMode.DoubleRow
E4NP = ml_dtypes.float8_e4m3

B, N, C = 8, 1024, 768
GD = 384
H, D = 6, 64
DP = 96                 # v head dim padded to 96 (dual-fp8 ldweights alignment); ones col at D
SCALE = D ** -0.5
HID = 3072
EPS = 1e-6
NH = 2                  # token n-halves of 512
NHW = N // NH           # 512
MC = N // 128           # 8 token chunks
CC = C // 128           # 6 feature chunks
GC = GD // 128          # 3 feature chunks per branch
JC = HID // 128         # 24 hidden chunks
WS = 1024.0             # weight quant scale (2^10)
QS = 2.0 ** -4          # q/k/v psum -> fp8 rescale (carries 2^6)
DQ_PROJ = 2.0 ** -16    # proj psum dequant (oT 2^6 * W 2^10)
DQ_FC = 2.0 ** -10      # fc psum dequant (acts true-scale, W 2^10)
EXP_SCALE_G = SCALE * 2.0 ** -12  # global: q,k each carry 2^6
EXP_SCALE_L = SCALE * 2.0 ** -20  # local: ql,kl each carry 2^10


def f32(ap):
    return ap.bitcast(F32)


def _build(flags):
    nc = bacc.Bacc("TRN2", target_bir_lowering=False, debug=False)

    x_d = nc.dram_tensor("x", (N, C), F32, kind="ExternalInput")
    gqk8_d = nc.dram_tensor("gqk8", (GD, 2 * GD), FP8, kind="ExternalInput")
    wv8_d = nc.dram_tensor("wv8", (GD, H * DP), FP8, kind="ExternalInput")
    lqkv8_d = nc.dram_tensor("lqkv8", (GD, 3 * GD), FP8, kind="ExternalInput")
    gp8_d = nc.dram_tensor("gp8", (GD, GD), FP8, kind="ExternalInput")
    lp8_d = nc.dram_tensor("lp8", (GD, GD), FP8, kind="ExternalInput")
    fc1h_d = nc.dram_tensor("fc1h", (C, HID), FP8, kind="ExternalInput")
    fc1l_d = nc.dram_tensor("fc1l", (C, HID), FP8, kind="ExternalInput")
    fc2h_d = nc.dram_tensor("fc2h", (HID, C), FP8, kind="ExternalInput")
    fc2l_d = nc.dram_tensor("fc2l", (HID, C), FP8, kind="ExternalInput")
    blkT_d = nc.dram_tensor("blkT_c", (H, GC * 128), mybir.dt.bfloat16,
                            kind="ExternalInput")
    blkT96_d = nc.dram_tensor("blkT96_c", (96, 3 * GC * 128), mybir.dt.bfloat16,
                              kind="ExternalInput")
    sumInd_d = nc.dram_tensor("sumInd_c", (96, H), mybir.dt.bfloat16,
                              kind="ExternalInput")
    opt = {}
    for nm, sz, fl in (("ln1_g", GD, "gb1g"), ("ln1_b", GD, "gb1g"),
                       ("ln1l_g", GD, "gb1l"), ("ln1l_b", GD, "gb1l"),
                       ("ln2_g", C, "gb2"), ("ln2_b", C, "gb2"),
                       ("g_proj_b", GD, "bias_gproj"), ("l_proj_b", GD, "bias_lproj"),
                       ("fc1_b", HID, "bias_fc1"), ("fc2_b", C, "bias_fc2")):
        if flags[fl]:
            opt[nm] = nc.dram_tensor(nm, (sz,), F32, kind="ExternalInput")
    out_d = nc.dram_tensor("out", (N, C), F32, kind="ExternalOutput")

    gqk8_v = gqk8_d.rearrange("(kc p) c -> p kc c", p=128)
    wv8_v = wv8_d.rearrange("(kc p) c -> p kc c", p=128)
    lqkv8_v = lqkv8_d.rearrange("(kc p) c -> p kc c", p=128)
    gp8_v = gp8_d.rearrange("(kc p) c -> p kc c", p=128)
    lp8_v = lp8_d.rearrange("(kc p) c -> p kc c", p=128)
    fc1h_v = fc1h_d.rearrange("(kc p) c -> p kc c", p=128)
    fc1l_v = fc1l_d.rearrange("(kc p) c -> p kc c", p=128)
    fc2h_v = fc2h_d.rearrange("(kc p) c -> p kc c", p=128)
    fc2l_v = fc2l_d.rearrange("(kc p) c -> p kc c", p=128)

    with tile.TileContext(nc) as tc, ExitStack() as top:
        consts = top.enter_context(tc.tile_pool(name="consts", bufs=1))
        core = top.enter_context(tc.tile_pool(name="core", bufs=1))
        wpool = top.enter_context(tc.tile_pool(name="wpool", bufs=1))

        identF = consts.tile([128, 128], F32, tag="identF")
        make_identity(nc, identF)
        onesF = consts.tile([128, 1], F32, tag="onesF")
        nc.vector.memset(onesF, 1.0)
        onesR = consts.tile([128, 1], F32R, tag="onesR")
        nc.gpsimd.tensor_copy(out=onesR, in_=onesF)
        onesB2 = consts.tile([128, 1], BF16, tag="onesB2")
        nc.vector.memset(onesB2, 1.0)
        onesRow = consts.tile([1, 128], BF16, tag="onesRow")
        nc.vector.memset(onesRow, 1.0)
        c1row = consts.tile([1, 64], BF16, tag="c1row")
        nc.vector.memset(c1row, 1.0)
        eps_t = consts.tile([1, 1], F32, tag="eps")
        nc.vector.memset(eps_t, EPS)
        # blkS[p, kc, j]: headsum lhsT (1 if j == 2*kc + p//64)
        blkS = consts.tile([128, GC, 2 * GC], BF16, tag="blkS")
        nc.vector.memset(blkS, 0.0)
        for kc in range(GC):
            nc.vector.memset(blkS[0:64, kc, 2 * kc:2 * kc + 1], 1.0)
            nc.vector.memset(blkS[64:128, kc, 2 * kc + 1:2 * kc + 2], 1.0)
        # host-built broadcast/sum indicator constants (partition-base rules
        # forbid single-partition memsets at unaligned bases)
        blkT = consts.tile([H, GC, 128], BF16, tag="blkT")
        nc.sync.dma_start(blkT, blkT_d.rearrange("j (kc p) -> j kc p", p=128))
        blkT96 = consts.tile([96, 3, GC, 128], BF16, tag="blkT96")
        nc.sync.dma_start(blkT96, blkT96_d.rearrange(
            "r (si kc p) -> r si kc p", si=3, p=128))
        sumInd = consts.tile([96, H], BF16, tag="sumInd")
        nc.sync.dma_start(sumInd, sumInd_d[:, :])

        def load_vec(dram, n_elems, tag):
            t = consts.tile([128, n_elems // 128], F32, tag=tag)
            nc.sync.dma_start(t, dram.rearrange("(c p) -> p c", p=128))
            return t

        g1g = load_vec(opt["ln1_g"], GD, "g1g") if flags["gb1g"] else None
        g1b = load_vec(opt["ln1_b"], GD, "g1b") if flags["gb1g"] else None
        l1g = load_vec(opt["ln1l_g"], GD, "l1g") if flags["gb1l"] else None
        l1b = load_vec(opt["ln1l_b"], GD, "l1b") if flags["gb1l"] else None
        g2g = load_vec(opt["ln2_g"], C, "g2g") if flags["gb2"] else None
        g2b = load_vec(opt["ln2_b"], C, "g2b") if flags["gb2"] else None
        gpb = load_vec(opt["g_proj_b"], GD, "gpb") if flags["bias_gproj"] else None
        lpb = load_vec(opt["l_proj_b"], GD, "lpb") if flags["bias_lproj"] else None
        fc1b = load_vec(opt["fc1_b"], HID, "fc1b") if flags["bias_fc1"] else None
        fc2b = load_vec(opt["fc2_b"], C, "fc2b") if flags["bias_fc2"] else None

        # resident fp8 weights (DMA'd on the sync queue AFTER x, before use)
        gqk8 = wpool.tile([128, GC, 2 * GD], FP8, tag="gqk8")
        wv8 = wpool.tile([128, GC, H * DP], FP8, tag="wv8")
        lqkv8 = wpool.tile([128, GC, 3 * GD], FP8, tag="lqkv8")
        gp8 = wpool.tile([128, GC, GD], FP8, tag="gp8")
        lp8 = wpool.tile([128, GC, GD], FP8, tag="lp8")
        fc1h = wpool.tile([128, CC, HID], FP8, tag="fc1h")
        fc1l = wpool.tile([128, CC, HID], FP8, tag="fc1l")
        fc2h = wpool.tile([128, JC, C], FP8, tag="fc2h")
        fc2l = wpool.tile([128, JC, C], FP8, tag="fc2l")

        def dma_weights():
            nc.sync.dma_start(gqk8, gqk8_v)
            nc.sync.dma_start(wv8, wv8_v)
            nc.sync.dma_start(lqkv8, lqkv8_v)
            nc.sync.dma_start(gp8, gp8_v)
            nc.sync.dma_start(lp8, lp8_v)
            for kc in range(0, CC, 2):
                nc.sync.dma_start(fc1h[:, kc:kc + 2], fc1h_v[:, kc:kc + 2])
                nc.sync.dma_start(fc1l[:, kc:kc + 2], fc1l_v[:, kc:kc + 2])
            for kc in range(0, JC, 8):
                nc.sync.dma_start(fc2h[:, kc:kc + 8], fc2h_v[:, kc:kc + 8])
                nc.sync.dma_start(fc2l[:, kc:kc + 8], fc2l_v[:, kc:kc + 8])

        xT = core.tile([128, CC, N], F32R, tag="xT")   # residual, feature-major

        # ---------------- phase A: load x, transpose to feature-major --------
        x_v = x_d.rearrange("(mq two p) c -> mq p two c", p=128, two=2)
        with tc.tile_pool(name="xtok", bufs=4) as xtok_p, \
             tc.tile_pool(name="ps_tr0", bufs=3, space="PSUM") as ps_tr0:
            xts = []
            for mq in range(MC // 2):
                xt = xtok_p.tile([128, 2, C], F32, tag="xt", name=f"xt{mq}")
                nc.sync.dma_start(xt, x_v[mq])
                xts.append(xt)
            dma_weights()
            for mq in range(MC // 2):
                xtr = xts[mq]
                for half in range(2):
                    m = 2 * mq + half
                    for cq in range(CC // 2):
                        ps = ps_tr0.tile([128, 2, 128], F32, tag="tr")
                        for h2 in range(2):
                            c = 2 * cq + h2
                            nc.tensor.transpose(
                                ps[:, h2], xtr[:, half, c * 128:(c + 1) * 128], identF)
                        dst = xT[:, 2 * cq:2 * cq + 2, m * 128:(m + 1) * 128]
                        if (m + cq) % 2 == 0:
                            nc.vector.tensor_copy(dst, ps)
                        else:
                            nc.scalar.copy(dst, ps)

        # ---------------- feature-major LayerNorm helper ----------------
        def ln_feat(lo, hi, dst, gv, bv, sq_p, st_p, bc_p, tmp_p, sq_eng,
                    op2_alt=False, nhs=None):
            """dst[:, c-lo, :] = fp8(LN(xT rows [lo*128, hi*128)) over features)."""
            nch = hi - lo
            inv = 1.0 / (nch * 128)
            for nh in (range(NH) if nhs is None else nhs):
                ns = slice(nh * NHW, (nh + 1) * NHW)
                st = st_p.tile([1, 2 * NHW], F32, tag="stat")
                for i, c in enumerate(range(lo, hi)):
                    nc.tensor.matmul(st[:, 0:NHW], onesR[:, 0:1], xT[:, c, ns],
                                     start=(i == 0), stop=(i == nch - 1))
                for i, c in enumerate(range(lo, hi)):
                    sq = sq_p.tile([128, NHW], BF16, tag="sq")
                    if sq_eng == "act":
                        nc.scalar.activation(sq, f32(xT[:, c, ns]), AF.Square)
                    else:
                        nc.gpsimd.tensor_tensor(sq, f32(xT[:, c, ns]),
                                                f32(xT[:, c, ns]), ALU.mult)
                    nc.tensor.matmul(st[:, NHW:2 * NHW], onesB2[:, 0:1], sq,
                                     start=(i == 0), stop=(i == nch - 1))
                # fall through: stats chain on DVE, normalize DVE(op1)+Pool(op2)
                mean = sq_p.tile([1, NHW], F32, tag="mean")
                nc.vector.tensor_scalar_mul(mean, st[:, 0:NHW], inv)
                e2 = sq_p.tile([1, NHW], F32, tag="e2")
                nc.vector.tensor_scalar_mul(e2, st[:, NHW:2 * NHW], inv)
                var = sq_p.tile([1, NHW], F32, tag="var")
                nc.vector.tensor_tensor(var, mean, mean, ALU.mult)
                nc.vector.tensor_tensor(var, e2, var, ALU.subtract)
                sr = sq_p.tile([1, NHW], F32, tag="sr")
                nc.scalar.activation(sr, var, AF.Sqrt, bias=eps_t[0:1, :], scale=1.0)
                r_bf = sq_p.tile([1, NHW], BF16, tag="r_bf")
                with nc.allow_low_precision(reason="bf16 rstd for bcast matmul"):
                    nc.vector.reciprocal(r_bf, sr)
                mr_bf = sq_p.tile([1, NHW], BF16, tag="mr_bf")
                nc.vector.tensor_tensor(mr_bf, mean, r_bf, ALU.mult)
                rB = bc_p.tile([128, NHW], F32, tag="rB")
                nc.tensor.matmul(rB, onesRow, r_bf, start=True, stop=True)
                mrB = bc_p.tile([128, NHW], F32, tag="mrB")
                nc.tensor.matmul(mrB, onesRow, mr_bf, start=True, stop=True)
                mrB_sb = sq_p.tile([128, NHW], BF16, tag="mrB_sb")
                nc.scalar.copy(mrB_sb, mrB)
                for c in range(lo, hi):
                    t = tmp_p.tile([128, NHW], BF16, tag="xnorm")
                    nc.vector.tensor_tensor(t, f32(xT[:, c, ns]), rB, ALU.mult)
                    dslice = dst[:, c - lo, ns]
                    eng2 = nc.vector if (op2_alt and c % 2 == 0) else nc.gpsimd
                    if gv is not None:
                        t2 = tmp_p.tile([128, NHW], BF16, tag="xnorm2")
                        eng2.tensor_tensor(t2, t, mrB_sb, ALU.subtract)
                        eng2.tensor_scalar(dslice, t2, gv[:, c - lo:c - lo + 1],
                                           bv[:, c - lo:c - lo + 1],
                                           ALU.mult, ALU.add)
                    else:
                        eng2.tensor_tensor(dslice, t, mrB_sb, ALU.subtract)

        # ---------------- phase B: LN1 (both branches) ----------------
        xgln = core.tile([128, GC, N], FP8, tag="xgln")
        xlln = core.tile([128, GC, N], FP8, tag="xlln")
        with tc.tile_pool(name="sq1", bufs=2) as sq_p, \
             tc.tile_pool(name="tmp1", bufs=2) as tmp_p, \
             tc.tile_pool(name="st1", bufs=1, space="PSUM") as st_p, \
             tc.tile_pool(name="bc1", bufs=2, space="PSUM") as bc_p:
            ln_feat(0, GC, xgln, g1g, g1b, sq_p, st_p, bc_p, tmp_p, "act")
            ln_feat(GC, CC, xlln, l1g, l1b, sq_p, st_p, bc_p, tmp_p, "act")

        # DR contraction helper over GC=3 chunks: pair (0,1) + single 2
        def mm3(ps, w, rhs_t, cols, ns):
            nc.tensor.matmul(ps, w[:, 0:2, cols], rhs_t[:, 0:2, ns],
                             start=True, stop=False, perf_mode=DR)
            nc.tensor.matmul(ps, w[:, 2, cols], rhs_t[:, 2, ns],
                             start=False, stop=True)

        # ---------------- phase C: all qkv projections (global + local) ------
        qT = core.tile([128, GC, N], FP8, tag="qT")      # x2^6
        kT = core.tile([128, GC, N], FP8, tag="kT")      # x2^6
        vpad = core.tile([128, MC, H * DP], FP8, tag="vpad")  # x2^6, ones col
        oT = core.tile([128, GC, N], FP8, tag="oT")      # x2^6
        qlT = core.tile([128, GC, N], BF16, tag="qlT")   # x2^10
        klT = core.tile([128, GC, N], BF16, tag="klT")   # x2^10
        vlT = core.tile([128, GC, N], FP8, tag="vlT")    # x2^6
        oTl = core.tile([128, GC, N], FP8, tag="oTl")    # x2^6
        prod_m = core.tile([128, GC, N], BF16, tag="prodm")
        prod_0 = core.tile([128, GC, N], BF16, tag="prod0")
        prod_p = core.tile([128, GC, N], BF16, tag="prodp")

        # ---------------- phase D: global attention (local qkv+prods dripped)
        drip_q = []

        def prod_unit(which):
            def go():
                if which == 0:
                    nc.vector.memset(prod_m[:, :, 0:1], 0.0)
                    nc.vector.tensor_tensor(prod_m[:, :, 1:N], qlT[:, :, 1:N],
                                            klT[:, :, 0:N - 1], ALU.mult)
                elif which == 1:
                    nc.vector.tensor_tensor(prod_0, qlT, klT, ALU.mult)
                else:
                    nc.vector.memset(prod_p[:, :, N - 1:N], 0.0)
                    nc.vector.tensor_tensor(prod_p[:, :, 0:N - 1],
                                            qlT[:, :, 0:N - 1],
                                            klT[:, :, 1:N], ALU.mult)
            return go

        def drip(n):
            for _ in range(n):
                if drip_q:
                    drip_q.pop(0)()

        with tc.tile_pool(name="esb", bufs=3) as e_p, \
             tc.tile_pool(name="small", bufs=3) as sm_p, \
             tc.tile_pool(name="plq", bufs=1, space="PSUM") as plq_p, \
             tc.tile_pool(name="pv", bufs=1, space="PSUM") as pv_p, \
             tc.tile_pool(name="psc", bufs=2, space="PSUM") as ps_p, \
             tc.tile_pool(name="po", bufs=1, space="PSUM") as po_p, \
             tc.tile_pool(name="pb", bufs=1, space="PSUM") as pb_p:

            # all global q,k then V (shared scope with the loop: no pool barrier)
            for mo in (0, GC, 1, GC + 1, 2, GC + 2):
                dst = qT if mo < GC else kT
                for nh in range(NH):
                    ns = slice(nh * NHW, (nh + 1) * NHW)
                    ps = plq_p.tile([128, NHW], F32, tag="lq", name="gqk")
                    mm3(ps, gqk8, xgln, slice(mo * 128, (mo + 1) * 128), ns)
                    nc.vector.tensor_scalar_mul(dst[:, mo % GC, ns], ps, QS)
            vpad_v = vpad.rearrange("p m (h e) -> p m h e", e=DP)
            hw_half = H * DP // 2
            for m in range(MC):
                for vh in range(2):
                    vs = slice(vh * hw_half, (vh + 1) * hw_half)
                    ps = pv_p.tile([128, hw_half], F32, tag="pv")
                    nc.tensor.matmul(ps, xgln[:, 0:2, m * 128:(m + 1) * 128],
                                     wv8[:, 0:2, vs], start=True, stop=False,
                                     perf_mode=DR)
                    nc.tensor.matmul(ps, xgln[:, 2, m * 128:(m + 1) * 128],
                                     wv8[:, 2, vs], start=False, stop=True)
                    nc.vector.tensor_scalar_mul(vpad[:, m, vs], ps, QS)
                nc.vector.memset(vpad_v[:, m, :, D:D + 1], 1.0)

            def lq_unit(pi, oc, nh):
                def go():
                    ns = slice(nh * NHW, (nh + 1) * NHW)
                    ps = plq_p.tile([128, NHW], F32, tag="lq")
                    mm3(ps, lqkv8, xlln,
                        slice(pi * GD + oc * 128, pi * GD + (oc + 1) * 128), ns)
                    if pi == 0:
                        nc.vector.tensor_copy(qlT[:, oc, ns], ps)
                    elif pi == 1:
                        nc.vector.tensor_copy(klT[:, oc, ns], ps)
                    else:
                        nc.vector.tensor_scalar_mul(vlT[:, oc, ns], ps, QS)
                return go

            for pi in (1, 0, 2):
                for oc in range(GC):
                    for nh in range(NH):
                        drip_q.append(lq_unit(pi, oc, nh))
            for which in range(3):
                drip_q.append(prod_unit(which))
            # scores -> exp -> DoubleRow AV -> per-(head, n-half) softmax
            for h in range(H):
                hc, hp = h // 2, (h % 2) * 64
                for nh in range(NH):
                    ns = slice(nh * NHW, (nh + 1) * NHW)
                    po = po_p.tile([DP, NHW], F32, tag="po")
                    for mp in range(MC // 2):
                        ps = ps_p.tile([128, 2, NHW], F32, tag="ps")
                        for half in range(2):
                            m = 2 * mp + half
                            nc.tensor.matmul(ps[:, half],
                                             kT[hp:hp + 64, hc, m * 128:(m + 1) * 128],
                                             qT[hp:hp + 64, hc, ns],
                                             start=True, stop=True)
                        e_sb = e_p.tile([128, 2, NHW], FP8, tag="e")
                        nc.scalar.activation(
                            e_sb.rearrange("p a b -> p (a b)"),
                            ps.rearrange("p a b -> p (a b)"), AF.Exp,
                            scale=EXP_SCALE_G)
                        nc.tensor.matmul(po,
                                         vpad[:, 2 * mp:2 * mp + 2,
                                              h * DP:(h + 1) * DP],
                                         e_sb, start=(mp == 0),
                                         stop=(mp == MC // 2 - 1), perf_mode=DR)
                    rcp = sm_p.tile([1, NHW], BF16, tag="rcp")
                    with nc.allow_low_precision(reason="bf16 recip for bcast"):
                        nc.vector.reciprocal(rcp, po[D:D + 1, :])
                    pb = pb_p.tile([64, NHW], F32, tag="pb")
                    nc.tensor.matmul(pb, c1row, rcp, start=True, stop=True)
                    pb_sb = sm_p.tile([64, NHW], BF16, tag="pbsb")
                    nc.vector.tensor_copy(pb_sb, pb)
                    nc.vector.tensor_tensor(oT[hp:hp + 64, hc, ns], po[0:D, :],
                                            pb_sb, ALU.mult)
                    drip(2)
            drip(len(drip_q))

        # ---------------- phase E: projections + local attention, nh-major ---
        o_un = core.tile([128, GC, N], BF16, tag="o_un")
        with tc.tile_pool(name="ltmp", bufs=3) as lt_p, \
             tc.tile_pool(name="pesc", bufs=1, space="PSUM") as pe_p, \
             tc.tile_pool(name="pdsum", bufs=1, space="PSUM") as pd_p, \
             tc.tile_pool(name="pab", bufs=2, space="PSUM") as pa_p, \
             tc.tile_pool(name="ppr", bufs=2, space="PSUM") as pp_p:

            def proj(w8, src, dst_row0, bias, mo, ns):
                ps = pp_p.tile([128, NHW], F32, tag="ppr")
                mm3(ps, w8, src, slice(mo * 128, (mo + 1) * 128), ns)
                row = dst_row0 + mo
                if bias is not None:
                    nc.scalar.activation(ps, ps, AF.Identity,
                                         bias=bias[:, mo:mo + 1], scale=DQ_PROJ)
                    nc.vector.tensor_tensor(xT[:, row, ns], f32(xT[:, row, ns]),
                                            ps, ALU.add)
                else:
                    nc.vector.scalar_tensor_tensor(
                        xT[:, row, ns], ps, DQ_PROJ, f32(xT[:, row, ns]),
                        ALU.mult, ALU.add)

            nc.vector.memset(o_un[:, :, 0:1], 0.0)
            for nh in range(NH):
                ns = slice(nh * NHW, (nh + 1) * NHW)
                # global proj + residual into xT rows [0, GD)
                for mo in range(GC):
                    proj(gp8, oT, 0, gpb, mo, ns)
                # head-sums into esc_all [96, 512]: shift si at partition 32*si
                esc_all = pe_p.tile([96, NHW], F32, tag="escall")
                for si, prod in enumerate((prod_m, prod_0, prod_p)):
                    for kc in range(GC):
                        nc.tensor.matmul(esc_all[32 * si:32 * si + H, :],
                                         blkS[:, kc, :], prod[:, kc, ns],
                                         start=(kc == 0), stop=(kc == GC - 1))
                if nh == 0:
                    nc.vector.memset(esc_all[0:H, 0:1], -1e30)
                if nh == NH - 1:
                    nc.vector.memset(esc_all[64:64 + H, NHW - 1:NHW], -1e30)
                ee_all = lt_p.tile([96, NHW], BF16, tag="ee_all")
                nc.vector.memset(ee_all, 0.0)
                for si in range(3):
                    nc.scalar.activation(ee_all[32 * si:32 * si + H, :],
                                         esc_all[32 * si:32 * si + H, :],
                                         AF.Exp, scale=EXP_SCALE_L)
                dsum = pd_p.tile([H, NHW], F32, tag="dsum")
                nc.tensor.matmul(dsum, sumInd, ee_all, start=True, stop=True)
                rr = lt_p.tile([H, NHW], BF16, tag="rr")
                with nc.allow_low_precision(reason="bf16 softmax recip"):
                    nc.vector.reciprocal(rr, dsum)
                # unnormalized o accumulation: eB broadcast via PE, v shifted
                lo_n, hi_n = nh * NHW, (nh + 1) * NHW
                for si in (0, 2, 1):
                    for kc in range(GC):
                        eB = pa_p.tile([128, NHW], F32, tag="eB")
                        nc.tensor.matmul(eB, blkT96[:, si, kc, :], ee_all,
                                         start=True, stop=True)
                        if si == 0:
                            vs, os_, oe = max(lo_n, 1) - 1, max(lo_n, 1), hi_n
                        elif si == 2:
                            vs, os_, oe = lo_n + 1, lo_n, min(hi_n, N - 1)
                        else:
                            vs, os_, oe = lo_n, lo_n, hi_n
                        a_sl = eB[:, os_ - lo_n:oe - lo_n]
                        v_sl = vlT[:, kc, vs:vs + (oe - os_)]
                        eng = nc.vector
                        if si == 0:
                            eng.tensor_tensor(o_un[:, kc, os_:oe], v_sl, a_sl,
                                              ALU.mult)
                        else:
                            t = lt_p.tile([128, NHW], BF16, tag="avt")
                            eng.tensor_tensor(t[:, 0:oe - os_], v_sl, a_sl, ALU.mult)
                            eng.tensor_tensor(o_un[:, kc, os_:oe],
                                              o_un[:, kc, os_:oe],
                                              t[:, 0:oe - os_], ALU.add)
                # normalize at the end: oTl = o_un * broadcast(rr), fp8
                for kc in range(GC):
                    rB = pa_p.tile([128, NHW], F32, tag="eB", name="rB")
                    nc.tensor.matmul(rB, blkT[:, kc, :], rr, start=True, stop=True)
                    nc.gpsimd.tensor_tensor(oTl[:, kc, ns], o_un[:, kc, ns],
                                            rB, ALU.mult)
                # local proj + residual into xT rows [GD, C)
                for mo in range(GC):
                    proj(lp8, oTl, GC, lpb, mo, ns)

        # ---------------- phases F+G: per-half LN2 then MLP ----------------
        hT = core.tile([128, CC, N], FP8, tag="hT")
        with tc.tile_pool(name="gl", bufs=1) as gl_pool, \
             tc.tile_pool(name="otok", bufs=2) as otok_p, \
             tc.tile_pool(name="outT", bufs=1) as outT_p:
            gls = [gl_pool.tile([128, 2, NHW], FP8, tag=f"gl{jp}", name=f"gl{jp}")
                   for jp in range(JC // 2)]
            def ln2(nh):
                with tc.tile_pool(name="sq2", bufs=2) as sq_p, \
                     tc.tile_pool(name="tmp2", bufs=2) as tmp_p, \
                     tc.tile_pool(name="st2", bufs=1, space="PSUM") as st_p, \
                     tc.tile_pool(name="bc2", bufs=1, space="PSUM") as bc_p:
                    ln_feat(0, CC, hT, g2g, g2b, sq_p, st_p, bc_p, tmp_p, "act",
                            op2_alt=True, nhs=[nh])

            def fc1_nh(nh, pm_p):
                ns = slice(nh * NHW, (nh + 1) * NHW)
                for jp in range(JC // 2):
                    pm = pm_p.tile([128, 2, NHW], F32, tag="pm")
                    for half in range(2):
                        j = 2 * jp + half
                        js = slice(j * 128, (j + 1) * 128)
                        for t in range(CC // 2):
                            nc.tensor.matmul(pm[:, half],
                                             fc1h[:, 2 * t:2 * t + 2, js],
                                             hT[:, 2 * t:2 * t + 2, ns],
                                             start=(t == 0), stop=False,
                                             perf_mode=DR)
                        for t in range(CC // 2):
                            nc.tensor.matmul(pm[:, half],
                                             fc1l[:, 2 * t:2 * t + 2, js],
                                             hT[:, 2 * t:2 * t + 2, ns],
                                             start=False, stop=(t == CC // 2 - 1),
                                             perf_mode=DR)
                    gl = gls[jp]
                    if fc1b is not None:
                        for half in range(2):
                            j = 2 * jp + half
                            nc.scalar.activation(gl[:, half], pm[:, half], AF.Gelu,
                                                 bias=fc1b[:, j:j + 1], scale=DQ_FC)
                    else:
                        nc.scalar.activation(gl.rearrange("p a b -> p (a b)"),
                                             pm.rearrange("p a b -> p (a b)"),
                                             AF.Gelu, scale=DQ_FC)

            def fc2_out_nh(nh, pz_p, ps_tr3):
                ns = slice(nh * NHW, (nh + 1) * NHW)
                outT = outT_p.tile([128, CC, NHW], F32, tag="outT")
                for mo in range(CC):
                    cs = slice(mo * 128, (mo + 1) * 128)
                    zp = pz_p.tile([128, NHW], F32, tag="pz")
                    for jp in range(JC // 2):
                        nc.tensor.matmul(zp, fc2h[:, 2 * jp:2 * jp + 2, cs],
                                         gls[jp], start=(jp == 0), stop=False,
                                         perf_mode=DR)
                    for jp in range(JC // 2):
                        nc.tensor.matmul(zp, fc2l[:, 2 * jp:2 * jp + 2, cs],
                                         gls[jp], start=False,
                                         stop=(jp == JC // 2 - 1), perf_mode=DR)
                    if fc2b is not None:
                        nc.scalar.activation(zp, zp, AF.Identity,
                                             bias=fc2b[:, mo:mo + 1], scale=DQ_FC)
                        nc.vector.tensor_tensor(outT[:, mo], f32(xT[:, mo, ns]),
                                                zp, ALU.add)
                    else:
                        nc.vector.scalar_tensor_tensor(
                            outT[:, mo], zp, DQ_FC, f32(xT[:, mo, ns]),
                            ALU.mult, ALU.add)
                for mq in range(NHW // 128):
                    ot = otok_p.tile([128, C], F32, tag="ot")
                    for cq in range(CC // 2):
                        ps = ps_tr3.tile([128, 2, 128], F32, tag="tr3")
                        for half in range(2):
                            c = 2 * cq + half
                            nc.tensor.transpose(
                                ps[:, half], outT[:, c, mq * 128:(mq + 1) * 128],
                                identF)
                        dst = ot[:, 2 * cq * 128:(2 * cq + 2) * 128]
                        dst = dst.rearrange("p (a b) -> p a b", a=2)
                        if (mq + cq) % 2 == 0:
                            nc.vector.tensor_copy(dst, ps)
                        else:
                            nc.scalar.copy(dst, ps)
                    tok0 = nh * NHW + mq * 128
                    nc.sync.dma_start(out_d[tok0:tok0 + 128, :], ot)

            ln2(0)
            with tc.tile_pool(name="pm0", bufs=2, space="PSUM") as pm_p:
                fc1_nh(0, pm_p)
            ln2(1)
            with tc.tile_pool(name="pmz", bufs=2, space="PSUM") as pm_p, \
                 tc.tile_pool(name="pz", bufs=2, space="PSUM") as pz_p, \
                 tc.tile_pool(name="ps_tr3", bufs=2, space="PSUM") as ps_tr3:
                fc2_out_nh(0, pz_p, ps_tr3)
                fc1_nh(1, pm_p)
                fc2_out_nh(1, pz_p, ps_tr3)

    nc.compile()
    return nc


_NC_CACHE = {}


def _q8(w, s=WS):
    return np.clip(w.astype(np.float64) * s, -240.0, 240.0).astype(E4NP)


def _q8_split(w, s=WS):
    ws = np.clip(w.astype(np.float64) * s, -240.0, 240.0)
    hi = ws.astype(E4NP)
    lo = np.clip(ws - hi.astype(np.float64), -240.0, 240.0).astype(E4NP)
    return hi, lo


def _blkT():
    a = np.zeros((H, GC, 128), np.float32)
    for kc in range(GC):
        a[2 * kc, kc, 0:64] = 1.0
        a[2 * kc + 1, kc, 64:128] = 1.0
    return a.reshape(H, GC * 128).astype(ml_dtypes.bfloat16)


def _blkT96():
    a = np.zeros((96, 3, GC, 128), np.float32)
    for si in range(3):
        for kc in range(GC):
            a[32 * si + 2 * kc, si, kc, 0:64] = 1.0
            a[32 * si + 2 * kc + 1, si, kc, 64:128] = 1.0
    return a.reshape(96, 3 * GC * 128).astype(ml_dtypes.bfloat16)


def _sumInd():
    a = np.zeros((96, H), np.float32)
    for si in range(3):
        for j in range(H):
            a[32 * si + j, j] = 1.0
    return a.astype(ml_dtypes.bfloat16)


def kernel(**inputs):
    inp = {k: np.ascontiguousarray(np.asarray(v), dtype=np.float32)
           for k, v in inputs.items()}
    flags = {
        "gb1g": not (np.all(inp["ln1_g"] == 1.0) and np.all(inp["ln1_b"] == 0.0)),
        "gb1l": not (np.all(inp["ln1l_g"] == 1.0) and np.all(inp["ln1l_b"] == 0.0)),
        "gb2": not (np.all(inp["ln2_g"] == 1.0) and np.all(inp["ln2_b"] == 0.0)),
        "bias_gproj": bool(np.any(inp["g_proj_b"] != 0.0)),
        "bias_lproj": bool(np.any(inp["l_proj_b"] != 0.0)),
        "bias_fc1": bool(np.any(inp["fc1_b"] != 0.0)),
        "bias_fc2": bool(np.any(inp["fc2_b"] != 0.0)),
    }
    key = tuple(sorted(flags.items()))
    nc = _NC_CACHE.get(key)
    if nc is None:
        nc = _build(flags)
        _NC_CACHE[key] = nc

    g_qkv = inp["g_qkv_w"]
    wv = np.zeros((GD, H * DP), np.float32)
    wv.reshape(GD, H, DP)[:, :, :D] = g_qkv[:, 2 * GD:].reshape(GD, H, D)
    fc1h, fc1l = _q8_split(inp["fc1_w"])
    fc2h, fc2l = _q8_split(inp["fc2_w"])
    weights = {
        "gqk8": _q8(g_qkv[:, :2 * GD]),
        "wv8": _q8(wv),
        "lqkv8": _q8(inp["l_qkv_w"]),
        "gp8": _q8(inp["g_proj_w"]),
        "lp8": _q8(inp["l_proj_w"]),
        "fc1h": fc1h, "fc1l": fc1l, "fc2h": fc2h, "fc2l": fc2l,
        "blkT_c": _blkT(), "blkT96_c": _blkT96(), "sumInd_c": _sumInd(),
    }
    for nm, fl in (("ln1_g", "gb1g"), ("ln1_b", "gb1g"), ("ln1l_g", "gb1l"),
                   ("ln1l_b", "gb1l"), ("ln2_g", "gb2"), ("ln2_b", "gb2"),
                   ("g_proj_b", "bias_gproj"), ("l_proj_b", "bias_lproj"),
                   ("fc1_b", "bias_fc1"), ("fc2_b", "bias_fc2")):
        if flags[fl]:
            weights[nm] = inp[nm]

    x = inp["x"]
    in_maps = [dict(weights, x=np.ascontiguousarray(x[b])) for b in range(B)]
    res = run_bass_kernel_spmd(nc, in_maps, core_ids=list(range(B)))
    return np.stack([res.results[b]["out"] for b in range(B)]).astype(np.float32)


# revision 53
# speedup vs baseline: 1.0356x; 1.0356x over previous
"""Trainium2 Bass kernel for nn_Block_local (dual global/banded-local attention block).

Sharding: data-parallel, one batch element per NeuronCore (B=8, 8 cores).
Feature-major activations ([C,N]); fp8e4 DoubleRow matmuls for all
weight-contractions (weights quantized host-side, fc1/fc2 split hi+lo fp8),
bf16 scores, fp8 softmax/activation intermediates, feature-major banded local
attention (shifts are free-axis slices; no shift DMAs, no local transposes).
"""
import os
import numpy as np
import ml_dtypes

import concourse.bass as bass
import concourse.bacc as bacc
import concourse.mybir as mybir
import concourse.tile as tile
from concourse.bass_utils import run_bass_kernel_spmd
from concourse.masks import make_identity
from contextlib import ExitStack

F32 = mybir.dt.float32
F32R = mybir.dt.float32r
BF16 = mybir.dt.bfloat16
FP8 = mybir.dt.float8e4
AF = mybir.ActivationFunctionType
ALU = mybir.AluOpType
DR = mybir.MatmulPerfMode.DoubleRow
E4NP = ml_dtypes.float8_e4m3

B, N, C = 8, 1024, 768
GD = 384
H, D = 6, 64
DP = 96                 # v head dim padded to 96 (dual-fp8 ldweights alignment); ones col at D
SCALE = D ** -0.5
HID = 3072
EPS = 1e-6
NH = 2                  # token n-halves of 512
NHW = N // NH           # 512
MC = N // 128           # 8 token chunks
CC = C // 128           # 6 feature chunks
GC = GD // 128          # 3 feature chunks per branch
JC = HID // 128         # 24 hidden chunks
WS = 1024.0             # weight quant scale (2^10)
QS = 2.0 ** -4          # q/k/v psum -> fp8 rescale (carries 2^6)
DQ_PROJ = 2.0 ** -16    # proj psum dequant (oT 2^6 * W 2^10)
DQ_FC = 2.0 ** -10      # fc psum dequant (acts true-scale, W 2^10)
EXP_SCALE_G = SCALE * 2.0 ** -12  # global: q,k each carry 2^6
EXP_SCALE_L = SCALE * 2.0 ** -20  # local: ql,kl each carry 2^10


def f32(ap):
    return ap.bitcast(F32)


def _build(flags):
    nc = bacc.Bacc("TRN2", target_bir_lowering=False, debug=False)

    x_d = nc.dram_tensor("x", (N, C), F32, kind="ExternalInput")
    gqk8_d = nc.dram_tensor("gqk8", (GD, 2 * GD), FP8, kind="ExternalInput")
    wv8_d = nc.dram_tensor("wv8", (GD, H * DP), FP8, kind="ExternalInput")
    lqkv8_d = nc.dram_tensor("lqkv8", (GD, 3 * GD), FP8, kind="ExternalInput")
    gp8_d = nc.dram_tensor("gp8", (GD, GD), FP8, kind="ExternalInput")
    lp8_d = nc.dram_tensor("lp8", (GD, GD), FP8, kind="ExternalInput")
    fc1h_d = nc.dram_tensor("fc1h", (C, HID), FP8, kind="ExternalInput")
    fc1l_d = nc.dram_tensor("fc1l", (C, HID), FP8, kind="ExternalInput")
    fc2h_d = nc.dram_tensor("fc2h", (HID, C), FP8, kind="ExternalInput")
    fc2l_d = nc.dram_tensor("fc2l", (HID, C), FP8, kind="ExternalInput")
    blkT_d = nc.dram_tensor("blkT_c", (H, GC * 128), mybir.dt.bfloat16,
                            kind="ExternalInput")
    blkT96_d = nc.dram_tensor("blkT96_c", (96, 3 * GC * 128), mybir.dt.bfloat16,
                              kind="ExternalInput")
    sumInd_d = nc.dram_tensor("sumInd_c", (96, H), mybir.dt.bfloat16,
                              kind="ExternalInput")
    opt = {}
    for nm, sz, fl in (("ln1_g", GD, "gb1g"), ("ln1_b", GD, "gb1g"),
                       ("ln1l_g", GD, "gb1l"), ("ln1l_b", GD, "gb1l"),
                       ("ln2_g", C, "gb2"), ("ln2_b", C, "gb2"),
                       ("g_proj_b", GD, "bias_gproj"), ("l_proj_b", GD, "bias_lproj"),
                       ("fc1_b", HID, "bias_fc1"), ("fc2_b", C, "bias_fc2")):
        if flags[fl]:
            opt[nm] = nc.dram_tensor(nm, (sz,), F32, kind="ExternalInput")
    out_d = nc.dram_tensor("out", (N, C), F32, kind="ExternalOutput")

    gqk8_v = gqk8_d.rearrange("(kc p) c -> p kc c", p=128)
    wv8_v = wv8_d.rearrange("(kc p) c -> p kc c", p=128)
    lqkv8_v = lqkv8_d.rearrange("(kc p) c -> p kc c", p=128)
    gp8_v = gp8_d.rearrange("(kc p) c -> p kc c", p=128)
    lp8_v = lp8_d.rearrange("(kc p) c -> p kc c", p=128)
    fc1h_v = fc1h_d.rearrange("(kc p) c -> p kc c", p=128)
    fc1l_v = fc1l_d.rearrange("(kc p) c -> p kc c", p=128)
    fc2h_v = fc2h_d.rearrange("(kc p) c -> p kc c", p=128)
    fc2l_v = fc2l_d.rearrange("(kc p) c -> p kc c", p=128)

    with tile.TileContext(nc) as tc, ExitStack() as top:
        consts = top.enter_context(tc.tile_pool(name="consts", bufs=1))
        core = top.enter_context(tc.tile_pool(name="core", bufs=1))
        wpool = top.enter_context(tc.tile_pool(name="wpool", bufs=1))

        identF = consts.tile([128, 128], F32, tag="identF")
        make_identity(nc, identF)
        onesF = consts.tile([128, 1], F32, tag="onesF")
        nc.vector.memset(onesF, 1.0)
        onesR = consts.tile([128, 1], F32R, tag="onesR")
        nc.gpsimd.tensor_copy(out=onesR, in_=onesF)
        onesB2 = consts.tile([128, 1], BF16, tag="onesB2")
        nc.vector.memset(onesB2, 1.0)
        onesRow = consts.tile([1, 128], BF16, tag="onesRow")
        nc.vector.memset(onesRow, 1.0)
        c1row = consts.tile([1, 64], BF16, tag="c1row")
        nc.vector.memset(c1row, 1.0)
        eps_t = consts.tile([1, 1], F32, tag="eps")
        nc.vector.memset(eps_t, EPS)
        # blkS[p, kc, j]: headsum lhsT (1 if j == 2*kc + p//64)
        blkS = consts.tile([128, GC, 2 * GC], BF16, tag="blkS")
        nc.vector.memset(blkS, 0.0)
        for kc in range(GC):
            nc.vector.memset(blkS[0:64, kc, 2 * kc:2 * kc + 1], 1.0)
            nc.vector.memset(blkS[64:128, kc, 2 * kc + 1:2 * kc + 2], 1.0)
        # host-built broadcast/sum indicator constants (partition-base rules
        # forbid single-partition memsets at unaligned bases)
        blkT = consts.tile([H, GC, 128], BF16, tag="blkT")
        nc.sync.dma_start(blkT, blkT_d.rearrange("j (kc p) -> j kc p", p=128))
        blkT96 = consts.tile([96, 3, GC, 128], BF16, tag="blkT96")
        nc.sync.dma_start(blkT96, blkT96_d.rearrange(
            "r (si kc p) -> r si kc p", si=3, p=128))
        sumInd = consts.tile([96, H], BF16, tag="sumInd")
        nc.sync.dma_start(sumInd, sumInd_d[:, :])

        def load_vec(dram, n_elems, tag):
            t = consts.tile([128, n_elems // 128], F32, tag=tag)
            nc.sync.dma_start(t, dram.rearrange("(c p) -> p c", p=128))
            return t

        g1g = load_vec(opt["ln1_g"], GD, "g1g") if flags["gb1g"] else None
        g1b = load_vec(opt["ln1_b"], GD, "g1b") if flags["gb1g"] else None
        l1g = load_vec(opt["ln1l_g"], GD, "l1g") if flags["gb1l"] else None
        l1b = load_vec(opt["ln1l_b"], GD, "l1b") if flags["gb1l"] else None
        g2g = load_vec(opt["ln2_g"], C, "g2g") if flags["gb2"] else None
        g2b = load_vec(opt["ln2_b"], C, "g2b") if flags["gb2"] else None
        gpb = load_vec(opt["g_proj_b"], GD, "gpb") if flags["bias_gproj"] else None
        lpb = load_vec(opt["l_proj_b"], GD, "lpb") if flags["bias_lproj"] else None
        fc1b = load_vec(opt["fc1_b"], HID, "fc1b") if flags["bias_fc1"] else None
        fc2b = load_vec(opt["fc2_b"], C, "fc2b") if flags["bias_fc2"] else None

        # resident fp8 weights (DMA'd on the sync queue AFTER x, before use)
        gqk8 = wpool.tile([128, GC, 2 * GD], FP8, tag="gqk8")
        wv8 = wpool.tile([128, GC, H * DP], FP8, tag="wv8")
        lqkv8 = wpool.tile([128, GC, 3 * GD], FP8, tag="lqkv8")
        gp8 = wpool.tile([128, GC, GD], FP8, tag="gp8")
        lp8 = wpool.tile([128, GC, GD], FP8, tag="lp8")
        fc1h = wpool.tile([128, CC, HID], FP8, tag="fc1h")
        fc1l = wpool.tile([128, CC, HID], FP8, tag="fc1l")
        fc2h = wpool.tile([128, JC, C], FP8, tag="fc2h")
        fc2l = wpool.tile([128, JC, C], FP8, tag="fc2l")

        def dma_weights():
            nc.sync.dma_start(gqk8, gqk8_v)
            nc.sync.dma_start(wv8, wv8_v)
            nc.sync.dma_start(lqkv8, lqkv8_v)
            nc.sync.dma_start(gp8, gp8_v)
            nc.sync.dma_start(lp8, lp8_v)
            for kc in range(0, CC, 2):
                nc.sync.dma_start(fc1h[:, kc:kc + 2], fc1h_v[:, kc:kc + 2])
                nc.sync.dma_start(fc1l[:, kc:kc + 2], fc1l_v[:, kc:kc + 2])
            for kc in range(0, JC, 8):
                nc.sync.dma_start(fc2h[:, kc:kc + 8], fc2h_v[:, kc:kc + 8])
                nc.sync.dma_start(fc2l[:, kc:kc + 8], fc2l_v[:, kc:kc + 8])

        xT = core.tile([128, CC, N], F32R, tag="xT")   # residual, feature-major

        # ---------------- phase A: load x, transpose to feature-major --------
        x_v = x_d.rearrange("(mq two p) c -> mq p two c", p=128, two=2)
        with tc.tile_pool(name="xtok", bufs=4) as xtok_p, \
             tc.tile_pool(name="ps_tr0", bufs=3, space="PSUM") as ps_tr0:
            xts = []
            for mq in range(MC // 2):
                xt = xtok_p.tile([128, 2, C], F32, tag="xt", name=f"xt{mq}")
                nc.sync.dma_start(xt, x_v[mq])
                xts.append(xt)
            dma_weights()
            for mq in range(MC // 2):
                xtr = xts[mq]
                for half in range(2):
                    m = 2 * mq + half
                    for cq in range(CC // 2):
                        ps = ps_tr0.tile([128, 2, 128], F32, tag="tr")
                        for h2 in range(2):
                            c = 2 * cq + h2
                            nc.tensor.transpose(
                                ps[:, h2], xtr[:, half, c * 128:(c + 1) * 128], identF)
                        dst = xT[:, 2 * cq:2 * cq + 2, m * 128:(m + 1) * 128]
                        if (m + cq) % 2 == 0:
                            nc.vector.tensor_copy(dst, ps)
                        else:
                            nc.scalar.copy(dst, ps)

        # ---------------- feature-major LayerNorm helper ----------------
        def ln_feat(lo, hi, dst, gv, bv, sq_p, st_p, bc_p, tmp_p, sq_eng,
                    op2_alt=False, nhs=None):
            """dst[:, c-lo, :] = fp8(LN(xT rows [lo*128, hi*128)) over features)."""
            nch = hi - lo
            inv = 1.0 / (nch * 128)
            for nh in (range(NH) if nhs is None else nhs):
                ns = slice(nh * NHW, (nh + 1) * NHW)
                st = st_p.tile([1, 2 * NHW], F32, tag="stat")
                for i, c in enumerate(range(lo, hi)):
                    nc.tensor.matmul(st[:, 0:NHW], onesR[:, 0:1], xT[:, c, ns],
                                     start=(i == 0), stop=(i == nch - 1))
                for i, c in enumerate(range(lo, hi)):
                    sq = sq_p.tile([128, NHW], BF16, tag="sq")
                    if sq_eng == "act":
                        nc.scalar.activation(sq, f32(xT[:, c, ns]), AF.Square)
                    else:
                        nc.gpsimd.tensor_tensor(sq, f32(xT[:, c, ns]),
                                                f32(xT[:, c, ns]), ALU.mult)
                    nc.tensor.matmul(st[:, NHW:2 * NHW], onesB2[:, 0:1], sq,
                                     start=(i == 0), stop=(i == nch - 1))
                # fall through: stats chain on DVE, normalize DVE(op1)+Pool(op2)
                mean = sq_p.tile([1, NHW], F32, tag="mean")
                nc.vector.tensor_scalar_mul(mean, st[:, 0:NHW], inv)
                e2 = sq_p.tile([1, NHW], F32, tag="e2")
                nc.vector.tensor_scalar_mul(e2, st[:, NHW:2 * NHW], inv)
                var = sq_p.tile([1, NHW], F32, tag="var")
                nc.vector.tensor_tensor(var, mean, mean, ALU.mult)
                nc.vector.tensor_tensor(var, e2, var, ALU.subtract)
                sr = sq_p.tile([1, NHW], F32, tag="sr")
                nc.scalar.activation(sr, var, AF.Sqrt, bias=eps_t[0:1, :], scale=1.0)
                r_bf = sq_p.tile([1, NHW], BF16, tag="r_bf")
                with nc.allow_low_precision(reason="bf16 rstd for bcast matmul"):
                    nc.vector.reciprocal(r_bf, sr)
                mr_bf = sq_p.tile([1, NHW], BF16, tag="mr_bf")
                nc.vector.tensor_tensor(mr_bf, mean, r_bf, ALU.mult)
                rB = bc_p.tile([128, NHW], F32, tag="rB")
                nc.tensor.matmul(rB, onesRow, r_bf, start=True, stop=True)
                mrB = bc_p.tile([128, NHW], F32, tag="mrB")
                nc.tensor.matmul(mrB, onesRow, mr_bf, start=True, stop=True)
                mrB_sb = sq_p.tile([128, NHW], BF16, tag="mrB_sb")
                nc.scalar.copy(mrB_sb, mrB)
                for c in range(lo, hi):
                    t = tmp_p.tile([128, NHW], BF16, tag="xnorm")
                    nc.vector.tensor_tensor(t, f32(xT[:, c, ns]), rB, ALU.mult)
                    dslice = dst[:, c - lo, ns]
                    eng2 = nc.vector if (op2_alt and c % 2 == 0) else nc.gpsimd
                    if gv is not None:
                        t2 = tmp_p.tile([128, NHW], BF16, tag="xnorm2")
                        eng2.tensor_tensor(t2, t, mrB_sb, ALU.subtract)
                        eng2.tensor_scalar(dslice, t2, gv[:, c - lo:c - lo + 1],
                                           bv[:, c - lo:c - lo + 1],
                                           ALU.mult, ALU.add)
                    else:
                        eng2.tensor_tensor(dslice, t, mrB_sb, ALU.subtract)

        # ---------------- phase B: LN1 (both branches) ----------------
        xgln = core.tile([128, GC, N], FP8, tag="xgln")
        xlln = core.tile([128, GC, N], FP8, tag="xlln")
        with tc.tile_pool(name="sq1", bufs=2) as sq_p, \
             tc.tile_pool(name="tmp1", bufs=2) as tmp_p, \
             tc.tile_pool(name="st1", bufs=1, space="PSUM") as st_p, \
             tc.tile_pool(name="bc1", bufs=2, space="PSUM") as bc_p:
            ln_feat(0, GC, xgln, g1g, g1b, sq_p, st_p, bc_p, tmp_p, "act")
            ln_feat(GC, CC, xlln, l1g, l1b, sq_p, st_p, bc_p, tmp_p, "act")

        # DR contraction helper over GC=3 chunks: pair (0,1) + single 2
        def mm3(ps, w, rhs_t, cols, ns):
            nc.tensor.matmul(ps, w[:, 0:2, cols], rhs_t[:, 0:2, ns],
                             start=True, stop=False, perf_mode=DR)
            nc.tensor.matmul(ps, w[:, 2, cols], rhs_t[:, 2, ns],
                             start=False, stop=True)

        # ---------------- phase C: all qkv projections (global + local) ------
        qT = core.tile([128, GC, N], FP8, tag="qT")      # x2^6
        kT = core.tile([128, GC, N], FP8, tag="kT")      # x2^6
        vpad = core.tile([128, MC, H * DP], FP8, tag="vpad")  # x2^6, ones col
        oT = core.tile([128, GC, N], FP8, tag="oT")      # x2^6
        qlT = core.tile([128, GC, N], BF16, tag="qlT")   # x2^10
        klT = core.tile([128, GC, N], BF16, tag="klT")   # x2^10
        vlT = core.tile([128, GC, N], FP8, tag="vlT")    # x2^6
        oTl = core.tile([128, GC, N], FP8, tag="oTl")    # x2^6
        prod_m = core.tile([128, GC, N], BF16, tag="prodm")
        prod_0 = core.tile([128, GC, N], BF16, tag="prod0")
        prod_p = core.tile([128, GC, N], BF16, tag="prodp")

        with tc.tile_pool(name="pqk", bufs=2, space="PSUM") as pq_p:
            # all global q,k up front
            for mo in range(2 * GC):
                dst = qT if mo < GC else kT
                for nh in range(NH):
                    ns = slice(nh * NHW, (nh + 1) * NHW)
                    ps = pq_p.tile([128, NHW], F32, tag="pq")
                    mm3(ps, gqk8, xgln, slice(mo * 128, (mo + 1) * 128), ns)
                    nc.vector.tensor_scalar_mul(dst[:, mo % GC, ns], ps, QS)
            vpad_v = vpad.rearrange("p m (h e) -> p m h e", e=DP)
            for m in range(MC):
                ps = pq_p.tile([128, H * DP], F32, tag="pv")
                hw_half = H * DP // 2
                for vh in range(2):
                    vs = slice(vh * hw_half, (vh + 1) * hw_half)
                    nc.tensor.matmul(ps[:, vs], xgln[:, 0:2, m * 128:(m + 1) * 128],
                                     wv8[:, 0:2, vs], start=True, stop=False,
                                     perf_mode=DR)
                    nc.tensor.matmul(ps[:, vs], xgln[:, 2, m * 128:(m + 1) * 128],
                                     wv8[:, 2, vs], start=False, stop=True)
                nc.vector.tensor_scalar_mul(vpad[:, m, :], ps, QS)
                nc.vector.memset(vpad_v[:, m, :, D:D + 1], 1.0)

        # ---------------- phase D: global attention (local qkv+prods dripped)
        drip_q = []

        def prod_unit(which):
            def go():
                if which == 0:
                    nc.vector.memset(prod_m[:, :, 0:1], 0.0)
                    nc.vector.tensor_tensor(prod_m[:, :, 1:N], qlT[:, :, 1:N],
                                            klT[:, :, 0:N - 1], ALU.mult)
                elif which == 1:
                    nc.vector.tensor_tensor(prod_0, qlT, klT, ALU.mult)
                else:
                    nc.vector.memset(prod_p[:, :, N - 1:N], 0.0)
                    nc.vector.tensor_tensor(prod_p[:, :, 0:N - 1],
                                            qlT[:, :, 0:N - 1],
                                            klT[:, :, 1:N], ALU.mult)
            return go

        def drip(n):
            for _ in range(n):
                if drip_q:
                    drip_q.pop(0)()

        with tc.tile_pool(name="esb", bufs=3) as e_p, \
             tc.tile_pool(name="small", bufs=3) as sm_p, \
             tc.tile_pool(name="plq", bufs=1, space="PSUM") as plq_p, \
             tc.tile_pool(name="psc", bufs=2, space="PSUM") as ps_p, \
             tc.tile_pool(name="po", bufs=2, space="PSUM") as po_p, \
             tc.tile_pool(name="pb", bufs=1, space="PSUM") as pb_p:

            def lq_unit(pi, oc, nh):
                def go():
                    ns = slice(nh * NHW, (nh + 1) * NHW)
                    ps = plq_p.tile([128, NHW], F32, tag="lq")
                    mm3(ps, lqkv8, xlln,
                        slice(pi * GD + oc * 128, pi * GD + (oc + 1) * 128), ns)
                    if pi == 0:
                        nc.vector.tensor_copy(qlT[:, oc, ns], ps)
                    elif pi == 1:
                        nc.vector.tensor_copy(klT[:, oc, ns], ps)
                    else:
                        nc.vector.tensor_scalar_mul(vlT[:, oc, ns], ps, QS)
                return go

            for pi in (1, 0, 2):
                for oc in range(GC):
                    for nh in range(NH):
                        drip_q.append(lq_unit(pi, oc, nh))
            for which in range(3):
                drip_q.append(prod_unit(which))
            # scores -> exp -> DoubleRow AV -> per-(head, n-half) softmax
            for h in range(H):
                hc, hp = h // 2, (h % 2) * 64
                for nh in range(NH):
                    ns = slice(nh * NHW, (nh + 1) * NHW)
                    po = po_p.tile([DP, NHW], F32, tag="po")
                    for mp in range(MC // 2):
                        ps = ps_p.tile([128, 2, NHW], F32, tag="ps")
                        for half in range(2):
                            m = 2 * mp + half
                            nc.tensor.matmul(ps[:, half],
                                             kT[hp:hp + 64, hc, m * 128:(m + 1) * 128],
                                             qT[hp:hp + 64, hc, ns],
                                             start=True, stop=True)
                        e_sb = e_p.tile([128, 2, NHW], FP8, tag="e")
                        nc.scalar.activation(
                            e_sb.rearrange("p a b -> p (a b)"),
                            ps.rearrange("p a b -> p (a b)"), AF.Exp,
                            scale=EXP_SCALE_G)
                        nc.tensor.matmul(po,
                                         vpad[:, 2 * mp:2 * mp + 2,
                                              h * DP:(h + 1) * DP],
                                         e_sb, start=(mp == 0),
                                         stop=(mp == MC // 2 - 1), perf_mode=DR)
                    rcp = sm_p.tile([1, NHW], BF16, tag="rcp")
                    with nc.allow_low_precision(reason="bf16 recip for bcast"):
                        nc.vector.reciprocal(rcp, po[D:D + 1, :])
                    pb = pb_p.tile([64, NHW], F32, tag="pb")
                    nc.tensor.matmul(pb, c1row, rcp, start=True, stop=True)
                    pb_sb = sm_p.tile([64, NHW], BF16, tag="pbsb")
                    nc.vector.tensor_copy(pb_sb, pb)
                    nc.vector.tensor_tensor(oT[hp:hp + 64, hc, ns], po[0:D, :],
                                            pb_sb, ALU.mult)
                    drip(2)
            drip(len(drip_q))

        # ---------------- phase E: projections + local attention, nh-major ---
        o_un = core.tile([128, GC, N], BF16, tag="o_un")
        with tc.tile_pool(name="ltmp", bufs=3) as lt_p, \
             tc.tile_pool(name="pesc", bufs=1, space="PSUM") as pe_p, \
             tc.tile_pool(name="pdsum", bufs=1, space="PSUM") as pd_p, \
             tc.tile_pool(name="pab", bufs=2, space="PSUM") as pa_p, \
             tc.tile_pool(name="ppr", bufs=2, space="PSUM") as pp_p:

            def proj(w8, src, dst_row0, bias, mo, ns):
                ps = pp_p.tile([128, NHW], F32, tag="ppr")
                mm3(ps, w8, src, slice(mo * 128, (mo + 1) * 128), ns)
                row = dst_row0 + mo
                if bias is not None:
                    nc.scalar.activation(ps, ps, AF.Identity,
                                         bias=bias[:, mo:mo + 1], scale=DQ_PROJ)
                    nc.vector.tensor_tensor(xT[:, row, ns], f32(xT[:, row, ns]),
                                            ps, ALU.add)
                else:
                    nc.vector.scalar_tensor_tensor(
                        xT[:, row, ns], ps, DQ_PROJ, f32(xT[:, row, ns]),
                        ALU.mult, ALU.add)

            nc.vector.memset(o_un[:, :, 0:1], 0.0)
            for nh in range(NH):
                ns = slice(nh * NHW, (nh + 1) * NHW)
                # global proj + residual into xT rows [0, GD)
                for mo in range(GC):
                    proj(gp8, oT, 0, gpb, mo, ns)
                # head-sums into esc_all [96, 512]: shift si at partition 32*si
                esc_all = pe_p.tile([96, NHW], F32, tag="escall")
                for si, prod in enumerate((prod_m, prod_0, prod_p)):
                    for kc in range(GC):
                        nc.tensor.matmul(esc_all[32 * si:32 * si + H, :],
                                         blkS[:, kc, :], prod[:, kc, ns],
                                         start=(kc == 0), stop=(kc == GC - 1))
                if nh == 0:
                    nc.vector.memset(esc_all[0:H, 0:1], -1e30)
                if nh == NH - 1:
                    nc.vector.memset(esc_all[64:64 + H, NHW - 1:NHW], -1e30)
                ee_all = lt_p.tile([96, NHW], BF16, tag="ee_all")
                nc.vector.memset(ee_all, 0.0)
                for si in range(3):
                    nc.scalar.activation(ee_all[32 * si:32 * si + H, :],
                                         esc_all[32 * si:32 * si + H, :],
                                         AF.Exp, scale=EXP_SCALE_L)
                dsum = pd_p.tile([H, NHW], F32, tag="dsum")
                nc.tensor.matmul(dsum, sumInd, ee_all, start=True, stop=True)
                rr = lt_p.tile([H, NHW], BF16, tag="rr")
                with nc.allow_low_precision(reason="bf16 softmax recip"):
                    nc.vector.reciprocal(rr, dsum)
                # unnormalized o accumulation: eB broadcast via PE, v shifted
                lo_n, hi_n = nh * NHW, (nh + 1) * NHW
                for si in (0, 2, 1):
                    for kc in range(GC):
                        eB = pa_p.tile([128, NHW], F32, tag="eB")
                        nc.tensor.matmul(eB, blkT96[:, si, kc, :], ee_all,
                                         start=True, stop=True)
                        if si == 0:
                            vs, os_, oe = max(lo_n, 1) - 1, max(lo_n, 1), hi_n
                        elif si == 2:
                            vs, os_, oe = lo_n + 1, lo_n, min(hi_n, N - 1)
                        else:
                            vs, os_, oe = lo_n, lo_n, hi_n
                        a_sl = eB[:, os_ - lo_n:oe - lo_n]
                        v_sl = vlT[:, kc, vs:vs + (oe - os_)]
                        eng = nc.vector
                        if si == 0:
                            eng.tensor_tensor(o_un[:, kc, os_:oe], v_sl, a_sl,
                                              ALU.mult)
                        else:
                            t = lt_p.tile([128, NHW], BF16, tag="avt")
                            eng.tensor_tensor(t[:, 0:oe - os_], v_sl, a_sl, ALU.mult)
                            eng.tensor_tensor(o_un[:, kc, os_:oe],
                                              o_un[:, kc, os_:oe],
                                              t[:, 0:oe - os_], ALU.add)
                # normalize at the end: oTl = o_un * broadcast(rr), fp8
                for kc in range(GC):
                    rB = pa_p.tile([128, NHW], F32, tag="eB", name="rB")
                    nc.tensor.matmul(rB, blkT[:, kc, :], rr, start=True, stop=True)
                    nc.gpsimd.tensor_tensor(oTl[:, kc, ns], o_un[:, kc, ns],
                                            rB, ALU.mult)
                # local proj + residual into xT rows [GD, C)
                for mo in range(GC):
                    proj(lp8, oTl, GC, lpb, mo, ns)

        # ---------------- phases F+G: per-half LN2 then MLP ----------------
        hT = core.tile([128, CC, N], FP8, tag="hT")
        with tc.tile_pool(name="gl", bufs=1) as gl_pool, \
             tc.tile_pool(name="otok", bufs=2) as otok_p, \
             tc.tile_pool(name="outT", bufs=1) as outT_p:
            gls = [gl_pool.tile([128, 2, NHW], FP8, tag=f"gl{jp}", name=f"gl{jp}")
                   for jp in range(JC // 2)]
            def ln2(nh):
                with tc.tile_pool(name="sq2", bufs=2) as sq_p, \
                     tc.tile_pool(name="tmp2", bufs=2) as tmp_p, \
                     tc.tile_pool(name="st2", bufs=1, space="PSUM") as st_p, \
                     tc.tile_pool(name="bc2", bufs=1, space="PSUM") as bc_p:
                    ln_feat(0, CC, hT, g2g, g2b, sq_p, st_p, bc_p, tmp_p, "act",
                            op2_alt=True, nhs=[nh])

            def fc1_nh(nh, pm_p):
                ns = slice(nh * NHW, (nh + 1) * NHW)
                for jp in range(JC // 2):
                    pm = pm_p.tile([128, 2, NHW], F32, tag="pm")
                    for half in range(2):
                        j = 2 * jp + half
                        js = slice(j * 128, (j + 1) * 128)
                        for t in range(CC // 2):
                            nc.tensor.matmul(pm[:, half],
                                             fc1h[:, 2 * t:2 * t + 2, js],
                                             hT[:, 2 * t:2 * t + 2, ns],
                                             start=(t == 0), stop=False,
                                             perf_mode=DR)
                        for t in range(CC // 2):
                            nc.tensor.matmul(pm[:, half],
                                             fc1l[:, 2 * t:2 * t + 2, js],
                                             hT[:, 2 * t:2 * t + 2, ns],
                                             start=False, stop=(t == CC // 2 - 1),
                                             perf_mode=DR)
                    gl = gls[jp]
                    if fc1b is not None:
                        for half in range(2):
                            j = 2 * jp + half
                            nc.scalar.activation(gl[:, half], pm[:, half], AF.Gelu,
                                                 bias=fc1b[:, j:j + 1], scale=DQ_FC)
                    else:
                        nc.scalar.activation(gl.rearrange("p a b -> p (a b)"),
                                             pm.rearrange("p a b -> p (a b)"),
                                             AF.Gelu, scale=DQ_FC)

            def fc2_out_nh(nh, pz_p, ps_tr3):
                ns = slice(nh * NHW, (nh + 1) * NHW)
                outT = outT_p.tile([128, CC, NHW], F32, tag="outT")
                for mo in range(CC):
                    cs = slice(mo * 128, (mo + 1) * 128)
                    zp = pz_p.tile([128, NHW], F32, tag="pz")
                    for jp in range(JC // 2):
                        nc.tensor.matmul(zp, fc2h[:, 2 * jp:2 * jp + 2, cs],
                                         gls[jp], start=(jp == 0), stop=False,
                                         perf_mode=DR)
                    for jp in range(JC // 2):
                        nc.tensor.matmul(zp, fc2l[:, 2 * jp:2 * jp + 2, cs],
                                         gls[jp], start=False,
                                         stop=(jp == JC // 2 - 1), perf_mode=DR)
                    if fc2b is not None:
                        nc.scalar.activation(zp, zp, AF.Identity,
                                             bias=fc2b[:, mo:mo + 1], scale=DQ_FC)
                        nc.vector.tensor_tensor(outT[:, mo], f32(xT[:, mo, ns]),
                                                zp, ALU.add)
                    else:
                        nc.vector.scalar_tensor_tensor(
                            outT[:, mo], zp, DQ_FC, f32(xT[:, mo, ns]),
                            ALU.mult, ALU.add)
                for mq in range(NHW // 128):
                    ot = otok_p.tile([128, C], F32, tag="ot")
                    for cq in range(CC // 2):
                        ps = ps_tr3.tile([128, 2, 128], F32, tag="tr3")
                        for half in range(2):
                            c = 2 * cq + half
                            nc.tensor.transpose(
                                ps[:, half], outT[:, c, mq * 128:(mq + 1) * 128],
                                identF)
                        dst = ot[:, 2 * cq * 128:(2 * cq + 2) * 128]
                        dst = dst.rearrange("p (a b) -> p a b", a=2)
                        if (mq + cq) % 2 == 0:
                            nc.vector.tensor_copy(dst, ps)
                        else:
                            nc.scalar.copy(dst, ps)
                    tok0 = nh * NHW + mq * 128
                    nc.sync.dma_start(out_d[tok0:tok0 + 128, :], ot)

            ln2(0)
            with tc.tile_pool(name="pm0", bufs=2, space="PSUM") as pm_p:
                fc1_nh(0, pm_p)
            ln2(1)
            with tc.tile_pool(name="pmz", bufs=2, space="PSUM") as pm_p, \
                 tc.tile_pool(name="pz", bufs=2, space="PSUM") as pz_p, \
                 tc.tile_pool(name="ps_tr3", bufs=2, space="PSUM") as ps_tr3:
                fc2_out_nh(0, pz_p, ps_tr3)
                fc1_nh(1, pm_p)
                fc2_out_nh(1, pz_p, ps_tr3)

    nc.compile()
    return nc


_NC_CACHE = {}


def _q8(w, s=WS):
    return np.clip(w.astype(np.float64) * s, -240.0, 240.0).astype(E4NP)


def _q8_split(w, s=WS):
    ws = np.clip(w.astype(np.float64) * s, -240.0, 240.0)
    hi = ws.astype(E4NP)
    lo = np.clip(ws - hi.astype(np.float64), -240.0, 240.0).astype(E4NP)
    return hi, lo


def _blkT():
    a = np.zeros((H, GC, 128), np.float32)
    for kc in range(GC):
        a[2 * kc, kc, 0:64] = 1.0
        a[2 * kc + 1, kc, 64:128] = 1.0
    return a.reshape(H, GC * 128).astype(ml_dtypes.bfloat16)


def _blkT96():
    a = np.zeros((96, 3, GC, 128), np.float32)
    for si in range(3):
        for kc in range(GC):
            a[32 * si + 2 * kc, si, kc, 0:64] = 1.0
            a[32 * si + 2 * kc + 1, si, kc, 64:128] = 1.0
    return a.reshape(96, 3 * GC * 128).astype(ml_dtypes.bfloat16)


def _sumInd():
    a = np.zeros((96, H), np.float32)
    for si in range(3):
        for j in range(H):
            a[32 * si + j, j] = 1.0
    return a.astype(ml_dtypes.bfloat16)


def kernel(**inputs):
    inp = {k: np.ascontiguousarray(np.asarray(v), dtype=np.float32)
           for k, v in inputs.items()}
    flags = {
        "gb1g": not (np.all(inp["ln1_g"] == 1.0) and np.all(inp["ln1_b"] == 0.0)),
        "gb1l": not (np.all(inp["ln1l_g"] == 1.0) and np.all(inp["ln1l_b"] == 0.0)),
        "gb2": not (np.all(inp["ln2_g"] == 1.0) and np.all(inp["ln2_b"] == 0.0)),
        "bias_gproj": bool(np.any(inp["g_proj_b"] != 0.0)),
        "bias_lproj": bool(np.any(inp["l_proj_b"] != 0.0)),
        "bias_fc1": bool(np.any(inp["fc1_b"] != 0.0)),
        "bias_fc2": bool(np.any(inp["fc2_b"] != 0.0)),
    }
    key = tuple(sorted(flags.items()))
    nc = _NC_CACHE.get(key)
    if nc is None:
        nc = _build(flags)
        _NC_CACHE[key] = nc

    g_qkv = inp["g_qkv_w"]
    wv = np.zeros((GD, H * DP), np.float32)
    wv.reshape(GD, H, DP)[:, :, :D] = g_qkv[:, 2 * GD:].reshape(GD, H, D)
    fc1h, fc1l = _q8_split(inp["fc1_w"])
    fc2h, fc2l = _q8_split(inp["fc2_w"])
    weights = {
        "gqk8": _q8(g_qkv[:, :2 * GD]),
        "wv8": _q8(wv),
        "lqkv8": _q8(inp["l_qkv_w"]),
        "gp8": _q8(inp["g_proj_w"]),
        "lp8": _q8(inp["l_proj_w"]),
        "fc1h": fc1h, "fc1l": fc1l, "fc2h": fc2h, "fc2l": fc2l,
        "blkT_c": _blkT(), "blkT96_c": _blkT96(), "sumInd_c": _sumInd(),
    }
    for nm, fl in (("ln1_g", "gb1g"), ("ln1_b", "gb1g"), ("ln1l_g", "gb1l"),
                   ("ln1l_b", "gb1l"), ("ln2_g", "gb2"), ("ln2_b", "gb2"),
                   ("g_proj_b", "bias_gproj"), ("l_proj_b", "bias_lproj"),
                   ("fc1_b", "bias_fc1"), ("fc2_b", "bias_fc2")):
        if flags[fl]:
            weights[nm] = inp[nm]

    x = inp["x"]
    in_maps = [dict(weights, x=np.ascontiguousarray(x[b])) for b in range(B)]
    res = run_bass_kernel_spmd(nc, in_maps, core_ids=list(range(B)))
    return np.stack([res.results[b]["out"] for b in range(B)]).astype(np.float32)


# revision 55
# speedup vs baseline: 1.0451x; 1.0092x over previous
"""Trainium2 Bass kernel for nn_Block_local (dual global/banded-local attention block).

Sharding: data-parallel, one batch element per NeuronCore (B=8, 8 cores).
Feature-major activations ([C,N]); fp8e4 DoubleRow matmuls for all
weight-contractions (weights quantized host-side, fc1/fc2 split hi+lo fp8),
bf16 scores, fp8 softmax/activation intermediates, feature-major banded local
attention (shifts are free-axis slices; no shift DMAs, no local transposes).
"""
import os
import numpy as np
import ml_dtypes

import concourse.bass as bass
import concourse.bacc as bacc
import concourse.mybir as mybir
import concourse.tile as tile
from concourse.bass_utils import run_bass_kernel_spmd
from concourse.masks import make_identity
from contextlib import ExitStack

F32 = mybir.dt.float32
F32R = mybir.dt.float32r
BF16 = mybir.dt.bfloat16
FP8 = mybir.dt.float8e4
AF = mybir.ActivationFunctionType
ALU = mybir.AluOpType
DR = mybir.MatmulPerfMode.DoubleRow
E4NP = ml_dtypes.float8_e4m3

B, N, C = 8, 1024, 768
GD = 384
H, D = 6, 64
DP = 96                 # v head dim padded to 96 (dual-fp8 ldweights alignment); ones col at D
SCALE = D ** -0.5
HID = 3072
EPS = 1e-6
NH = 2                  # token n-halves of 512
NHW = N // NH           # 512
MC = N // 128           # 8 token chunks
CC = C // 128           # 6 feature chunks
GC = GD // 128          # 3 feature chunks per branch
JC = HID // 128         # 24 hidden chunks
WS = 1024.0             # weight quant scale (2^10)
QS = 2.0 ** -4          # q/k/v psum -> fp8 rescale (carries 2^6)
DQ_PROJ = 2.0 ** -16    # proj psum dequant (oT 2^6 * W 2^10)
DQ_FC = 2.0 ** -10      # fc psum dequant (acts true-scale, W 2^10)
EXP_SCALE_G = SCALE * 2.0 ** -12  # global: q,k each carry 2^6
EXP_SCALE_L = SCALE * 2.0 ** -20  # local: ql,kl each carry 2^10


def f32(ap):
    return ap.bitcast(F32)


def _build(flags):
    nc = bacc.Bacc("TRN2", target_bir_lowering=False, debug=False)

    x_d = nc.dram_tensor("x", (N, C), F32, kind="ExternalInput")
    gqk8_d = nc.dram_tensor("gqk8", (GD, 2 * GD), FP8, kind="ExternalInput")
    wv8_d = nc.dram_tensor("wv8", (GD, H * DP), FP8, kind="ExternalInput")
    lqkv8_d = nc.dram_tensor("lqkv8", (GD, 3 * GD), FP8, kind="ExternalInput")
    gp8_d = nc.dram_tensor("gp8", (GD, GD), FP8, kind="ExternalInput")
    lp8_d = nc.dram_tensor("lp8", (GD, GD), FP8, kind="ExternalInput")
    fc1h_d = nc.dram_tensor("fc1h", (C, HID), FP8, kind="ExternalInput")
    fc1l_d = nc.dram_tensor("fc1l", (C, HID), FP8, kind="ExternalInput")
    fc2h_d = nc.dram_tensor("fc2h", (HID, C), FP8, kind="ExternalInput")
    fc2l_d = nc.dram_tensor("fc2l", (HID, C), FP8, kind="ExternalInput")
    blkT_d = nc.dram_tensor("blkT_c", (H, GC * 128), mybir.dt.bfloat16,
                            kind="ExternalInput")
    blkT96_d = nc.dram_tensor("blkT96_c", (96, 3 * GC * 128), mybir.dt.bfloat16,
                              kind="ExternalInput")
    sumInd_d = nc.dram_tensor("sumInd_c", (96, H), mybir.dt.bfloat16,
                              kind="ExternalInput")
    opt = {}
    for nm, sz, fl in (("ln1_g", GD, "gb1g"), ("ln1_b", GD, "gb1g"),
                       ("ln1l_g", GD, "gb1l"), ("ln1l_b", GD, "gb1l"),
                       ("ln2_g", C, "gb2"), ("ln2_b", C, "gb2"),
                       ("g_proj_b", GD, "bias_gproj"), ("l_proj_b", GD, "bias_lproj"),
                       ("fc1_b", HID, "bias_fc1"), ("fc2_b", C, "bias_fc2")):
        if flags[fl]:
            opt[nm] = nc.dram_tensor(nm, (sz,), F32, kind="ExternalInput")
    out_d = nc.dram_tensor("out", (N, C), F32, kind="ExternalOutput")

    gqk8_v = gqk8_d.rearrange("(kc p) c -> p kc c", p=128)
    wv8_v = wv8_d.rearrange("(kc p) c -> p kc c", p=128)
    lqkv8_v = lqkv8_d.rearrange("(kc p) c -> p kc c", p=128)
    gp8_v = gp8_d.rearrange("(kc p) c -> p kc c", p=128)
    lp8_v = lp8_d.rearrange("(kc p) c -> p kc c", p=128)
    fc1h_v = fc1h_d.rearrange("(kc p) c -> p kc c", p=128)
    fc1l_v = fc1l_d.rearrange("(kc p) c -> p kc c", p=128)
    fc2h_v = fc2h_d.rearrange("(kc p) c -> p kc c", p=128)
    fc2l_v = fc2l_d.rearrange("(kc p) c -> p kc c", p=128)

    with tile.TileContext(nc) as tc, ExitStack() as top:
        consts = top.enter_context(tc.tile_pool(name="consts", bufs=1))
        core = top.enter_context(tc.tile_pool(name="core", bufs=1))
        wpool = top.enter_context(tc.tile_pool(name="wpool", bufs=1))

        identF = consts.tile([128, 128], F32, tag="identF")
        make_identity(nc, identF)
        onesF = consts.tile([128, 1], F32, tag="onesF")
        nc.vector.memset(onesF, 1.0)
        onesR = consts.tile([128, 1], F32R, tag="onesR")
        nc.gpsimd.tensor_copy(out=onesR, in_=onesF)
        onesB2 = consts.tile([128, 1], BF16, tag="onesB2")
        nc.vector.memset(onesB2, 1.0)
        onesRow = consts.tile([1, 128], BF16, tag="onesRow")
        nc.vector.memset(onesRow, 1.0)
        c1row = consts.tile([1, 64], BF16, tag="c1row")
        nc.vector.memset(c1row, 1.0)
        eps_t = consts.tile([1, 1], F32, tag="eps")
        nc.vector.memset(eps_t, EPS)
        # blkS[p, kc, j]: headsum lhsT (1 if j == 2*kc + p//64)
        blkS = consts.tile([128, GC, 2 * GC], BF16, tag="blkS")
        nc.vector.memset(blkS, 0.0)
        for kc in range(GC):
            nc.vector.memset(blkS[0:64, kc, 2 * kc:2 * kc + 1], 1.0)
            nc.vector.memset(blkS[64:128, kc, 2 * kc + 1:2 * kc + 2], 1.0)
        # host-built broadcast/sum indicator constants (partition-base rules
        # forbid single-partition memsets at unaligned bases)
        blkT = consts.tile([H, GC, 128], BF16, tag="blkT")
        nc.sync.dma_start(blkT, blkT_d.rearrange("j (kc p) -> j kc p", p=128))
        blkT96 = consts.tile([96, 3, GC, 128], BF16, tag="blkT96")
        nc.sync.dma_start(blkT96, blkT96_d.rearrange(
            "r (si kc p) -> r si kc p", si=3, p=128))
        sumInd = consts.tile([96, H], BF16, tag="sumInd")
        nc.sync.dma_start(sumInd, sumInd_d[:, :])

        def load_vec(dram, n_elems, tag):
            t = consts.tile([128, n_elems // 128], F32, tag=tag)
            nc.sync.dma_start(t, dram.rearrange("(c p) -> p c", p=128))
            return t

        g1g = load_vec(opt["ln1_g"], GD, "g1g") if flags["gb1g"] else None
        g1b = load_vec(opt["ln1_b"], GD, "g1b") if flags["gb1g"] else None
        l1g = load_vec(opt["ln1l_g"], GD, "l1g") if flags["gb1l"] else None
        l1b = load_vec(opt["ln1l_b"], GD, "l1b") if flags["gb1l"] else None
        g2g = load_vec(opt["ln2_g"], C, "g2g") if flags["gb2"] else None
        g2b = load_vec(opt["ln2_b"], C, "g2b") if flags["gb2"] else None
        gpb = load_vec(opt["g_proj_b"], GD, "gpb") if flags["bias_gproj"] else None
        lpb = load_vec(opt["l_proj_b"], GD, "lpb") if flags["bias_lproj"] else None
        fc1b = load_vec(opt["fc1_b"], HID, "fc1b") if flags["bias_fc1"] else None
        fc2b = load_vec(opt["fc2_b"], C, "fc2b") if flags["bias_fc2"] else None

        # resident fp8 weights (DMA'd on the sync queue AFTER x, before use)
        gqk8 = wpool.tile([128, GC, 2 * GD], FP8, tag="gqk8")
        wv8 = wpool.tile([128, GC, H * DP], FP8, tag="wv8")
        lqkv8 = wpool.tile([128, GC, 3 * GD], FP8, tag="lqkv8")
        gp8 = wpool.tile([128, GC, GD], FP8, tag="gp8")
        lp8 = wpool.tile([128, GC, GD], FP8, tag="lp8")
        fc1h = wpool.tile([128, CC, HID], FP8, tag="fc1h")
        fc1l = wpool.tile([128, CC, HID], FP8, tag="fc1l")
        fc2h = wpool.tile([128, JC, C], FP8, tag="fc2h")
        fc2l = wpool.tile([128, JC, C], FP8, tag="fc2l")

        def dma_weights():
            nc.sync.dma_start(gqk8, gqk8_v)
            nc.sync.dma_start(wv8, wv8_v)
            nc.sync.dma_start(lqkv8, lqkv8_v)
            nc.sync.dma_start(gp8, gp8_v)
            nc.sync.dma_start(lp8, lp8_v)
            for kc in range(0, CC, 2):
                nc.sync.dma_start(fc1h[:, kc:kc + 2], fc1h_v[:, kc:kc + 2])
                nc.sync.dma_start(fc1l[:, kc:kc + 2], fc1l_v[:, kc:kc + 2])
            for kc in range(0, JC, 8):
                nc.sync.dma_start(fc2h[:, kc:kc + 8], fc2h_v[:, kc:kc + 8])
                nc.sync.dma_start(fc2l[:, kc:kc + 8], fc2l_v[:, kc:kc + 8])

        xT = core.tile([128, CC, N], F32R, tag="xT")   # residual, feature-major

        # ---------------- phase A: load x, transpose to feature-major --------
        x_v = x_d.rearrange("(mq two p) c -> mq p two c", p=128, two=2)
        with tc.tile_pool(name="xtok", bufs=4) as xtok_p, \
             tc.tile_pool(name="ps_tr0", bufs=3, space="PSUM") as ps_tr0:
            xts = []
            for mq in range(MC // 2):
                xt = xtok_p.tile([128, 2, C], F32, tag="xt", name=f"xt{mq}")
                nc.sync.dma_start(xt, x_v[mq])
                xts.append(xt)
            dma_weights()
            for mq in range(MC // 2):
                xtr = xts[mq]
                for half in range(2):
                    m = 2 * mq + half
                    for cq in range(CC // 2):
                        ps = ps_tr0.tile([128, 2, 128], F32, tag="tr")
                        for h2 in range(2):
                            c = 2 * cq + h2
                            nc.tensor.transpose(
                                ps[:, h2], xtr[:, half, c * 128:(c + 1) * 128], identF)
                        dst = xT[:, 2 * cq:2 * cq + 2, m * 128:(m + 1) * 128]
                        if (m + cq) % 2 == 0:
                            nc.vector.tensor_copy(dst, ps)
                        else:
                            nc.scalar.copy(dst, ps)

        # ---------------- feature-major LayerNorm helper ----------------
        def ln_feat(lo, hi, dst, gv, bv, sq_p, st_p, bc_p, tmp_p, sq_eng,
                    op2_alt=False, nhs=None):
            """dst[:, c-lo, :] = fp8(LN(xT rows [lo*128, hi*128)) over features)."""
            nch = hi - lo
            inv = 1.0 / (nch * 128)
            for nh in (range(NH) if nhs is None else nhs):
                ns = slice(nh * NHW, (nh + 1) * NHW)
                st = st_p.tile([1, 2 * NHW], F32, tag="stat")
                for i, c in enumerate(range(lo, hi)):
                    nc.tensor.matmul(st[:, 0:NHW], onesR[:, 0:1], xT[:, c, ns],
                                     start=(i == 0), stop=(i == nch - 1))
                for i, c in enumerate(range(lo, hi)):
                    sq = sq_p.tile([128, NHW], BF16, tag="sq")
                    if sq_eng == "act":
                        nc.scalar.activation(sq, f32(xT[:, c, ns]), AF.Square)
                    else:
                        nc.gpsimd.tensor_tensor(sq, f32(xT[:, c, ns]),
                                                f32(xT[:, c, ns]), ALU.mult)
                    nc.tensor.matmul(st[:, NHW:2 * NHW], onesB2[:, 0:1], sq,
                                     start=(i == 0), stop=(i == nch - 1))
                # fall through: stats chain on DVE, normalize DVE(op1)+Pool(op2)
                mean = sq_p.tile([1, NHW], F32, tag="mean")
                nc.vector.tensor_scalar_mul(mean, st[:, 0:NHW], inv)
                e2 = sq_p.tile([1, NHW], F32, tag="e2")
                nc.vector.tensor_scalar_mul(e2, st[:, NHW:2 * NHW], inv)
                var = sq_p.tile([1, NHW], F32, tag="var")
                nc.vector.tensor_tensor(var, mean, mean, ALU.mult)
                nc.vector.tensor_tensor(var, e2, var, ALU.subtract)
                sr = sq_p.tile([1, NHW], F32, tag="sr")
                nc.scalar.activation(sr, var, AF.Sqrt, bias=eps_t[0:1, :], scale=1.0)
                r_bf = sq_p.tile([1, NHW], BF16, tag="r_bf")
                with nc.allow_low_precision(reason="bf16 rstd for bcast matmul"):
                    nc.vector.reciprocal(r_bf, sr)
                mr_bf = sq_p.tile([1, NHW], BF16, tag="mr_bf")
                nc.vector.tensor_tensor(mr_bf, mean, r_bf, ALU.mult)
                rB = bc_p.tile([128, NHW], F32, tag="rB")
                nc.tensor.matmul(rB, onesRow, r_bf, start=True, stop=True)
                mrB = bc_p.tile([128, NHW], F32, tag="mrB")
                nc.tensor.matmul(mrB, onesRow, mr_bf, start=True, stop=True)
                mrB_sb = sq_p.tile([128, NHW], BF16, tag="mrB_sb")
                nc.scalar.copy(mrB_sb, mrB)
                for c in range(lo, hi):
                    t = tmp_p.tile([128, NHW], BF16, tag="xnorm")
                    nc.vector.tensor_tensor(t, f32(xT[:, c, ns]), rB, ALU.mult)
                    dslice = dst[:, c - lo, ns]
                    eng2 = nc.vector if (op2_alt and c % 2 == 0) else nc.gpsimd
                    if gv is not None:
                        t2 = tmp_p.tile([128, NHW], BF16, tag="xnorm2")
                        eng2.tensor_tensor(t2, t, mrB_sb, ALU.subtract)
                        eng2.tensor_scalar(dslice, t2, gv[:, c - lo:c - lo + 1],
                                           bv[:, c - lo:c - lo + 1],
                                           ALU.mult, ALU.add)
                    else:
                        eng2.tensor_tensor(dslice, t, mrB_sb, ALU.subtract)

        # ---------------- phase B: LN1 (both branches) ----------------
        xgln = core.tile([128, GC, N], FP8, tag="xgln")
        xlln = core.tile([128, GC, N], FP8, tag="xlln")
        with tc.tile_pool(name="sq1", bufs=2) as sq_p, \
             tc.tile_pool(name="tmp1", bufs=2) as tmp_p, \
             tc.tile_pool(name="st1", bufs=1, space="PSUM") as st_p, \
             tc.tile_pool(name="bc1", bufs=2, space="PSUM") as bc_p:
            ln_feat(0, GC, xgln, g1g, g1b, sq_p, st_p, bc_p, tmp_p, "act")
            ln_feat(GC, CC, xlln, l1g, l1b, sq_p, st_p, bc_p, tmp_p, "act")

        # DR contraction helper over GC=3 chunks: pair (0,1) + single 2
        def mm3(ps, w, rhs_t, cols, ns):
            nc.tensor.matmul(ps, w[:, 0:2, cols], rhs_t[:, 0:2, ns],
                             start=True, stop=False, perf_mode=DR)
            nc.tensor.matmul(ps, w[:, 2, cols], rhs_t[:, 2, ns],
                             start=False, stop=True)

        # ---------------- phase C: all qkv projections (global + local) ------
        qT = core.tile([128, GC, N], FP8, tag="qT")      # x2^6
        kT = core.tile([128, GC, N], FP8, tag="kT")      # x2^6
        vpad = core.tile([128, MC, H * DP], FP8, tag="vpad")  # x2^6, ones col
        oT = core.tile([128, GC, N], FP8, tag="oT")      # x2^6
        qlT = core.tile([128, GC, N], BF16, tag="qlT")   # x2^10
        klT = core.tile([128, GC, N], BF16, tag="klT")   # x2^10
        vlT = core.tile([128, GC, N], FP8, tag="vlT")    # x2^6
        oTl = core.tile([128, GC, N], FP8, tag="oTl")    # x2^6
        prod_m = core.tile([128, GC, N], BF16, tag="prodm")
        prod_0 = core.tile([128, GC, N], BF16, tag="prod0")
        prod_p = core.tile([128, GC, N], BF16, tag="prodp")

        with tc.tile_pool(name="pqk", bufs=2, space="PSUM") as pq_p:
            # all global q,k up front
            for mo in range(2 * GC):
                dst = qT if mo < GC else kT
                for nh in range(NH):
                    ns = slice(nh * NHW, (nh + 1) * NHW)
                    ps = pq_p.tile([128, NHW], F32, tag="pq")
                    mm3(ps, gqk8, xgln, slice(mo * 128, (mo + 1) * 128), ns)
                    nc.vector.tensor_scalar_mul(dst[:, mo % GC, ns], ps, QS)
            vpad_v = vpad.rearrange("p m (h e) -> p m h e", e=DP)
            for m in range(MC):
                ps = pq_p.tile([128, H * DP], F32, tag="pv")
                hw_half = H * DP // 2
                for vh in range(2):
                    vs = slice(vh * hw_half, (vh + 1) * hw_half)
                    nc.tensor.matmul(ps[:, vs], xgln[:, 0:2, m * 128:(m + 1) * 128],
                                     wv8[:, 0:2, vs], start=True, stop=False,
                                     perf_mode=DR)
                    nc.tensor.matmul(ps[:, vs], xgln[:, 2, m * 128:(m + 1) * 128],
                                     wv8[:, 2, vs], start=False, stop=True)
                nc.vector.tensor_scalar_mul(vpad[:, m, :], ps, QS)
                nc.vector.memset(vpad_v[:, m, :, D:D + 1], 1.0)

        # ---------------- phase D: global attention (local qkv+prods dripped)
        drip_q = []

        def prod_unit(which):
            def go():
                if which == 0:
                    nc.vector.memset(prod_m[:, :, 0:1], 0.0)
                    nc.vector.tensor_tensor(prod_m[:, :, 1:N], qlT[:, :, 1:N],
                                            klT[:, :, 0:N - 1], ALU.mult)
                elif which == 1:
                    nc.vector.tensor_tensor(prod_0, qlT, klT, ALU.mult)
                else:
                    nc.vector.memset(prod_p[:, :, N - 1:N], 0.0)
                    nc.vector.tensor_tensor(prod_p[:, :, 0:N - 1],
                                            qlT[:, :, 0:N - 1],
                                            klT[:, :, 1:N], ALU.mult)
            return go

        def drip(n):
            for _ in range(n):
                if drip_q:
                    drip_q.pop(0)()

        with tc.tile_pool(name="esb", bufs=3) as e_p, \
             tc.tile_pool(name="small", bufs=3) as sm_p, \
             tc.tile_pool(name="plq", bufs=1, space="PSUM") as plq_p, \
             tc.tile_pool(name="psc", bufs=2, space="PSUM") as ps_p, \
             tc.tile_pool(name="po", bufs=2, space="PSUM") as po_p, \
             tc.tile_pool(name="pb", bufs=1, space="PSUM") as pb_p:

            def lq_unit(pi, oc, nh):
                def go():
                    ns = slice(nh * NHW, (nh + 1) * NHW)
                    ps = plq_p.tile([128, NHW], F32, tag="lq")
                    mm3(ps, lqkv8, xlln,
                        slice(pi * GD + oc * 128, pi * GD + (oc + 1) * 128), ns)
                    if pi == 0:
                        nc.vector.tensor_copy(qlT[:, oc, ns], ps)
                    elif pi == 1:
                        nc.vector.tensor_copy(klT[:, oc, ns], ps)
                    else:
                        nc.vector.tensor_scalar_mul(vlT[:, oc, ns], ps, QS)
                return go

            for pi in (1, 0, 2):
                for oc in range(GC):
                    for nh in range(NH):
                        drip_q.append(lq_unit(pi, oc, nh))
            for which in range(3):
                drip_q.append(prod_unit(which))
            # scores -> exp -> DoubleRow AV -> per-(head, n-half) softmax
            for h in range(H):
                hc, hp = h // 2, (h % 2) * 64
                for nh in range(NH):
                    ns = slice(nh * NHW, (nh + 1) * NHW)
                    po = po_p.tile([DP, NHW], F32, tag="po")
                    for mp in range(MC // 2):
                        ps = ps_p.tile([128, 2, NHW], F32, tag="ps")
                        for half in range(2):
                            m = 2 * mp + half
                            nc.tensor.matmul(ps[:, half],
                                             kT[hp:hp + 64, hc, m * 128:(m + 1) * 128],
                                             qT[hp:hp + 64, hc, ns],
                                             start=True, stop=True)
                        e_sb = e_p.tile([128, 2, NHW], FP8, tag="e")
                        nc.scalar.activation(
                            e_sb.rearrange("p a b -> p (a b)"),
                            ps.rearrange("p a b -> p (a b)"), AF.Exp,
                            scale=EXP_SCALE_G)
                        nc.tensor.matmul(po,
                                         vpad[:, 2 * mp:2 * mp + 2,
                                              h * DP:(h + 1) * DP],
                                         e_sb, start=(mp == 0),
                                         stop=(mp == MC // 2 - 1), perf_mode=DR)
                    rcp = sm_p.tile([1, NHW], BF16, tag="rcp")
                    with nc.allow_low_precision(reason="bf16 recip for bcast"):
                        nc.vector.reciprocal(rcp, po[D:D + 1, :])
                    pb = pb_p.tile([64, NHW], F32, tag="pb")
                    nc.tensor.matmul(pb, c1row, rcp, start=True, stop=True)
                    pb_sb = sm_p.tile([64, NHW], BF16, tag="pbsb")
                    nc.vector.tensor_copy(pb_sb, pb)
                    nc.vector.tensor_tensor(oT[hp:hp + 64, hc, ns], po[0:D, :],
                                            pb_sb, ALU.mult)
                    drip(2)
            drip(len(drip_q))

        # ---------------- phase E: projections + local attention, nh-major ---
        o_un = core.tile([128, GC, N], BF16, tag="o_un")
        with tc.tile_pool(name="ltmp", bufs=3) as lt_p, \
             tc.tile_pool(name="pesc", bufs=1, space="PSUM") as pe_p, \
             tc.tile_pool(name="pdsum", bufs=1, space="PSUM") as pd_p, \
             tc.tile_pool(name="pab", bufs=2, space="PSUM") as pa_p, \
             tc.tile_pool(name="ppr", bufs=2, space="PSUM") as pp_p:

            def proj(w8, src, dst_row0, bias, mo, ns):
                ps = pp_p.tile([128, NHW], F32, tag="ppr")
                mm3(ps, w8, src, slice(mo * 128, (mo + 1) * 128), ns)
                row = dst_row0 + mo
                if bias is not None:
                    nc.scalar.activation(ps, ps, AF.Identity,
                                         bias=bias[:, mo:mo + 1], scale=DQ_PROJ)
                    nc.vector.tensor_tensor(xT[:, row, ns], f32(xT[:, row, ns]),
                                            ps, ALU.add)
                else:
                    nc.vector.scalar_tensor_tensor(
                        xT[:, row, ns], ps, DQ_PROJ, f32(xT[:, row, ns]),
                        ALU.mult, ALU.add)

            nc.vector.memset(o_un[:, :, 0:1], 0.0)
            for nh in range(NH):
                ns = slice(nh * NHW, (nh + 1) * NHW)
                # global proj + residual into xT rows [0, GD)
                for mo in range(GC):
                    proj(gp8, oT, 0, gpb, mo, ns)
                # head-sums into esc_all [96, 512]: shift si at partition 32*si
                esc_all = pe_p.tile([96, NHW], F32, tag="escall")
                for si, prod in enumerate((prod_m, prod_0, prod_p)):
                    for kc in range(GC):
                        nc.tensor.matmul(esc_all[32 * si:32 * si + H, :],
                                         blkS[:, kc, :], prod[:, kc, ns],
                                         start=(kc == 0), stop=(kc == GC - 1))
                if nh == 0:
                    nc.vector.memset(esc_all[0:H, 0:1], -1e30)
                if nh == NH - 1:
                    nc.vector.memset(esc_all[64:64 + H, NHW - 1:NHW], -1e30)
                ee_all = lt_p.tile([96, NHW], BF16, tag="ee_all")
                nc.vector.memset(ee_all, 0.0)
                for si in range(3):
                    nc.scalar.activation(ee_all[32 * si:32 * si + H, :],
                                         esc_all[32 * si:32 * si + H, :],
                                         AF.Exp, scale=EXP_SCALE_L)
                dsum = pd_p.tile([H, NHW], F32, tag="dsum")
                nc.tensor.matmul(dsum, sumInd, ee_all, start=True, stop=True)
                rr = lt_p.tile([H, NHW], BF16, tag="rr")
                with nc.allow_low_precision(reason="bf16 softmax recip"):
                    nc.vector.reciprocal(rr, dsum)
                # unnormalized o accumulation: eB broadcast via PE, v shifted
                lo_n, hi_n = nh * NHW, (nh + 1) * NHW
                for si in (0, 2, 1):
                    for kc in range(GC):
                        eB = pa_p.tile([128, NHW], F32, tag="eB")
                        nc.tensor.matmul(eB, blkT96[:, si, kc, :], ee_all,
                                         start=True, stop=True)
                        if si == 0:
                            vs, os_, oe = max(lo_n, 1) - 1, max(lo_n, 1), hi_n
                        elif si == 2:
                            vs, os_, oe = lo_n + 1, lo_n, min(hi_n, N - 1)
                        else:
                            vs, os_, oe = lo_n, lo_n, hi_n
                        a_sl = eB[:, os_ - lo_n:oe - lo_n]
                        v_sl = vlT[:, kc, vs:vs + (oe - os_)]
                        eng = nc.vector
                        if si == 0:
                            eng.tensor_tensor(o_un[:, kc, os_:oe], v_sl, a_sl,
                                              ALU.mult)
                        else:
                            t = lt_p.tile([128, NHW], BF16, tag="avt")
                            eng.tensor_tensor(t[:, 0:oe - os_], v_sl, a_sl, ALU.mult)
                            eng.tensor_tensor(o_un[:, kc, os_:oe],
                                              o_un[:, kc, os_:oe],
                                              t[:, 0:oe - os_], ALU.add)
                # normalize at the end: oTl = o_un * broadcast(rr), fp8
                for kc in range(GC):
                    rB = pa_p.tile([128, NHW], F32, tag="eB", name="rB")
                    nc.tensor.matmul(rB, blkT[:, kc, :], rr, start=True, stop=True)
                    nc.gpsimd.tensor_tensor(oTl[:, kc, ns], o_un[:, kc, ns],
                                            rB, ALU.mult)
                # local proj + residual into xT rows [GD, C)
                for mo in range(GC):
                    proj(lp8, oTl, GC, lpb, mo, ns)

        # ---------------- phases F+G: per-half LN2 then MLP ----------------
        hT = core.tile([128, CC, N], FP8, tag="hT")
        with tc.tile_pool(name="gl", bufs=1) as gl_pool, \
             tc.tile_pool(name="otok", bufs=3) as otok_p, \
             tc.tile_pool(name="outT", bufs=1) as outT_p:
            gls = [gl_pool.tile([128, 2, NHW], FP8, tag=f"gl{jp}", name=f"gl{jp}")
                   for jp in range(JC // 2)]
            def ln2(nh):
                with tc.tile_pool(name="sq2", bufs=2) as sq_p, \
                     tc.tile_pool(name="tmp2", bufs=2) as tmp_p, \
                     tc.tile_pool(name="st2", bufs=1, space="PSUM") as st_p, \
                     tc.tile_pool(name="bc2", bufs=1, space="PSUM") as bc_p:
                    ln_feat(0, CC, hT, g2g, g2b, sq_p, st_p, bc_p, tmp_p, "act",
                            op2_alt=True, nhs=[nh])

            def fc1_nh(nh, pm_p):
                ns = slice(nh * NHW, (nh + 1) * NHW)
                for jp in range(JC // 2):
                    pm = pm_p.tile([128, 2, NHW], F32, tag="pm")
                    for half in range(2):
                        j = 2 * jp + half
                        js = slice(j * 128, (j + 1) * 128)
                        for t in range(CC // 2):
                            nc.tensor.matmul(pm[:, half],
                                             fc1h[:, 2 * t:2 * t + 2, js],
                                             hT[:, 2 * t:2 * t + 2, ns],
                                             start=(t == 0), stop=False,
                                             perf_mode=DR)
                        for t in range(CC // 2):
                            nc.tensor.matmul(pm[:, half],
                                             fc1l[:, 2 * t:2 * t + 2, js],
                                             hT[:, 2 * t:2 * t + 2, ns],
                                             start=False, stop=(t == CC // 2 - 1),
                                             perf_mode=DR)
                    gl = gls[jp]
                    if fc1b is not None:
                        for half in range(2):
                            j = 2 * jp + half
                            nc.scalar.activation(gl[:, half], pm[:, half], AF.Gelu,
                                                 bias=fc1b[:, j:j + 1], scale=DQ_FC)
                    else:
                        nc.scalar.activation(gl.rearrange("p a b -> p (a b)"),
                                             pm.rearrange("p a b -> p (a b)"),
                                             AF.Gelu, scale=DQ_FC)

            def fc2_out_nh(nh, pz_p, ps_tr3):
                ns = slice(nh * NHW, (nh + 1) * NHW)
                outT = outT_p.tile([128, CC, NHW], F32, tag="outT")
                for mo in range(CC):
                    cs = slice(mo * 128, (mo + 1) * 128)
                    zp = pz_p.tile([128, NHW], F32, tag="pz")
                    for jp in range(JC // 2):
                        nc.tensor.matmul(zp, fc2h[:, 2 * jp:2 * jp + 2, cs],
                                         gls[jp], start=(jp == 0), stop=False,
                                         perf_mode=DR)
                    for jp in range(JC // 2):
                        nc.tensor.matmul(zp, fc2l[:, 2 * jp:2 * jp + 2, cs],
                                         gls[jp], start=False,
                                         stop=(jp == JC // 2 - 1), perf_mode=DR)
                    if fc2b is not None:
                        nc.scalar.activation(zp, zp, AF.Identity,
                                             bias=fc2b[:, mo:mo + 1], scale=DQ_FC)
                        nc.vector.tensor_tensor(outT[:, mo], f32(xT[:, mo, ns]),
                                                zp, ALU.add)
                    else:
                        nc.vector.scalar_tensor_tensor(
                            outT[:, mo], zp, DQ_FC, f32(xT[:, mo, ns]),
                            ALU.mult, ALU.add)
                for mq in range(NHW // 128):
                    ot = otok_p.tile([128, C], F32, tag="ot")
                    for cq in range(CC // 2):
                        ps = ps_tr3.tile([128, 2, 128], F32, tag="tr3")
                        for half in range(2):
                            c = 2 * cq + half
                            nc.tensor.transpose(
                                ps[:, half], outT[:, c, mq * 128:(mq + 1) * 128],
                                identF)
                        dst = ot[:, 2 * cq * 128:(2 * cq + 2) * 128]
                        dst = dst.rearrange("p (a b) -> p a b", a=2)
                        if (mq + cq) % 2 == 0:
                            nc.vector.tensor_copy(dst, ps)
                        else:
                            nc.scalar.copy(dst, ps)
                    tok0 = nh * NHW + mq * 128
                    nc.sync.dma_start(out_d[tok0:tok0 + 128, :], ot)

            ln2(0)
            with tc.tile_pool(name="pm0", bufs=2, space="PSUM") as pm_p:
                fc1_nh(0, pm_p)
            ln2(1)
            with tc.tile_pool(name="pmz", bufs=2, space="PSUM") as pm_p, \
                 tc.tile_pool(name="pz", bufs=2, space="PSUM") as pz_p, \
                 tc.tile_pool(name="ps_tr3", bufs=2, space="PSUM") as ps_tr3:
                fc2_out_nh(0, pz_p, ps_tr3)
                fc1_nh(1, pm_p)
                fc2_out_nh(1, pz_p, ps_tr3)

    nc.compile()
    return nc


_NC_CACHE = {}


def _q8(w, s=WS):
    return np.clip(w.astype(np.float64) * s, -240.0, 240.0).astype(E4NP)


def _q8_split(w, s=WS):
    ws = np.clip(w.astype(np.float64) * s, -240.0, 240.0)
    hi = ws.astype(E4NP)
    lo = np.clip(ws - hi.astype(np.float64), -240.0, 240.0).astype(E4NP)
    return hi, lo


def _blkT():
    a = np.zeros((H, GC, 128), np.float32)
    for kc in range(GC):
        a[2 * kc, kc, 0:64] = 1.0
        a[2 * kc + 1, kc, 64:128] = 1.0
    return a.reshape(H, GC * 128).astype(ml_dtypes.bfloat16)


def _blkT96():
    a = np.zeros((96, 3, GC, 128), np.float32)
    for si in range(3):
        for kc in range(GC):
            a[32 * si + 2 * kc, si, kc, 0:64] = 1.0
            a[32 * si + 2 * kc + 1, si, kc, 64:128] = 1.0
    return a.reshape(96, 3 * GC * 128).astype(ml_dtypes.bfloat16)


def _sumInd():
    a = np.zeros((96, H), np.float32)
    for si in range(3):
        for j in range(H):
            a[32 * si + j, j] = 1.0
    return a.astype(ml_dtypes.bfloat16)


def kernel(**inputs):
    inp = {k: np.ascontiguousarray(np.asarray(v), dtype=np.float32)
           for k, v in inputs.items()}
    flags = {
        "gb1g": not (np.all(inp["ln1_g"] == 1.0) and np.all(inp["ln1_b"] == 0.0)),
        "gb1l": not (np.all(inp["ln1l_g"] == 1.0) and np.all(inp["ln1l_b"] == 0.0)),
        "gb2": not (np.all(inp["ln2_g"] == 1.0) and np.all(inp["ln2_b"] == 0.0)),
        "bias_gproj": bool(np.any(inp["g_proj_b"] != 0.0)),
        "bias_lproj": bool(np.any(inp["l_proj_b"] != 0.0)),
        "bias_fc1": bool(np.any(inp["fc1_b"] != 0.0)),
        "bias_fc2": bool(np.any(inp["fc2_b"] != 0.0)),
    }
    key = tuple(sorted(flags.items()))
    nc = _NC_CACHE.get(key)
    if nc is None:
        nc = _build(flags)
        _NC_CACHE[key] = nc

    g_qkv = inp["g_qkv_w"]
    wv = np.zeros((GD, H * DP), np.float32)
    wv.reshape(GD, H, DP)[:, :, :D] = g_qkv[:, 2 * GD:].reshape(GD, H, D)
    fc1h, fc1l = _q8_split(inp["fc1_w"])
    fc2h, fc2l = _q8_split(inp["fc2_w"])
    weights = {
        "gqk8": _q8(g_qkv[:, :2 * GD]),
        "wv8": _q8(wv),
        "lqkv8": _q8(inp["l_qkv_w"]),
        "gp8": _q8(inp["g_proj_w"]),
        "lp8": _q8(inp["l_proj_w"]),
        "fc1h": fc1h, "fc1l": fc1l, "fc2h": fc2h, "fc2l": fc2l,
        "blkT_c": _blkT(), "blkT96_c": _blkT96(), "sumInd_c": _sumInd(),
    }
    for nm, fl in (("ln1_g", "gb1g"), ("ln1_b", "gb1g"), ("ln1l_g", "gb1l"),
                   ("ln1l_b", "gb1l"), ("ln2_g", "gb2"), ("ln2_b", "gb2"),
                   ("g_proj_b", "bias_gproj"), ("l_proj_b", "bias_lproj"),
                   ("fc1_b", "bias_fc1"), ("fc2_b", "bias_fc2")):
        if flags[fl]:
            weights[nm] = inp[nm]

    x = inp["x"]
    in_maps = [dict(weights, x=np.ascontiguousarray(x[b])) for b in range(B)]
    res = run_bass_kernel_spmd(nc, in_maps, core_ids=list(range(B)))
    return np.stack([res.results[b]["out"] for b in range(B)]).astype(np.float32)
